# revision 40
# baseline (speedup 1.0000x reference)
"""Additive (Bahdanau-style) attention kernel for Trainium2, 8 NeuronCores.

reference computation (per batch b of 32, T=1024 timesteps, H=1024):
    mlp_hidden = selu([enc[b,t]; hid[b]] @ W1 + b1)     # (T, H)
    scores     = mlp_hidden @ W2 + b2                   # (T, 1)
    weights    = softmax(scores over t)
    out[b]     = sum_t weights[t] * enc[b,t]            # (H,)

Distribution: data-parallel over batch, 4 batches per core, no collectives.

Per-core algorithm (v3, fp8 DoubleRow, software-pipelined):
  - the hid @ W1[H:] + b1 term is per-batch constant; precomputed on the
    host and shipped as per-(j,b) bias columns (relu / exp variants).
  - scores path in fp8e4: E loaded once from HBM as bf16 (DMA cast),
    PE-transposed per 128x128 block, PSUM->SBUF copy casts to fp8.  W1a
    host-cast to fp8 with a 2^10 scale (entries ~1e-2 sit below fp8's
    normal range); the descale rides the ACT/DVE affine inputs.  The mlp
    matmul uses perf_mode=DoubleRow: one instruction contracts 256 rows.
  - selu = max(x,0) + min(alpha*e^x, alpha) (+const, dropped by softmax).
    exp always on ACT; the relu alternates between ACT (even j) and DVE
    tensor_scalar (odd j) to keep the j-loop from being ACT-bound.  For
    DVE j's the whole s2 is scaled by SW (host pre-scales that j's bias
    by SW and W2 column by 1/SW) so no extra scaling op is needed.
  - eT cast-copies alternate DVE / ACT per k for the same reason.
  - scores dot / context matmul have 1-wide outputs; packed 4-way into
    disjoint 32-column PE groups via tile_position (stationaries zero-
    padded to 32 cols so all PSUM partitions are written), then reduced
    across the 4 partial rows by a masked-ones PE matmul to partition 0.
  - software pipeline per batch b: loads(b+1) up front; transpose group
    tt of batch b+1 emitted after main-matmul group j=tt; epilogue(b-1)
    pieces emitted after j=4/5/6 so its matmuls land mid-stream (PE warm,
    no HAM re-throttle); score tail at the end of the j-loop.
  - softmax: exp off the reduced score rows (scores O(1), no max shift);
    1/Z folded into the output copy's scale.
"""

import math

import ml_dtypes
import numpy as np

import concourse.tile as tile
from concourse.masks import make_identity
from concourse import bacc, mybir
from concourse.bass_utils import run_bass_kernel_spmd

F32 = mybir.dt.float32
BF16 = mybir.dt.bfloat16
FP8 = mybir.dt.float8e4
ALU = mybir.AluOpType
ACTF = mybir.ActivationFunctionType
DR = mybir.MatmulPerfMode.DoubleRow

N_CORES = 8
B = 32
T = 1024
H = 1024
BL = B // N_CORES          # batches per core = 4
KC = H // 128              # contraction chunks = 8
JC = H // 128              # hidden-unit chunks = 8
TT = T // 128              # 128-row t-subtiles per batch = 8

SELU_LAMBDA = 1.0507009873554805
SELU_ALPHA = 1.6732632423543772
SW = 1024.0                # fp8 scale for W1a (and odd-j s2 scale)


def build_kernel():
    nc = bacc.Bacc("TRN2", target_bir_lowering=False, debug=False,
                   num_devices=N_CORES)

    enc = nc.dram_tensor("enc", [BL, TT, 128, H], F32, kind="ExternalInput").ap()
    w1a8 = nc.dram_tensor("w1a8", [KC, 128, H], FP8, kind="ExternalInput").ap()
    w2lp = nc.dram_tensor("w2lp", [128, JC, 32], BF16, kind="ExternalInput").ap()
    hbe = nc.dram_tensor("hbe", [128, JC, BL], F32, kind="ExternalInput").ap()
    hbr = nc.dram_tensor("hbr", [128, JC, BL], F32, kind="ExternalInput").ap()
    maskb = nc.dram_tensor("maskb", [128, 2], BF16, kind="ExternalInput").ap()
    wcz = nc.dram_tensor("wcz", [128, KC, 32], BF16, kind="ExternalInput").ap()
    outp4 = nc.dram_tensor("outp4", [BL, 4, 512], F32,
                           kind="ExternalOutput").ap()
    zs = nc.dram_tensor("zs", [BL, 2], F32, kind="ExternalOutput").ap()

    with tile.TileContext(nc) as tc:
        with (
            tc.tile_pool(name="consts", bufs=1) as consts,
            tc.tile_pool(name="encp", bufs=3) as encp,
            tc.tile_pool(name="etp", bufs=2) as etp,
            tc.tile_pool(name="selu", bufs=4) as selup,
            tc.tile_pool(name="score", bufs=2) as scorep,
            tc.tile_pool(name="outp", bufs=2) as outp,
            tc.tile_pool(name="psum", bufs=2, space="PSUM") as psum,
        ):
            # identity + PE warmup first: the warmup keeps the TensorE
            # activity monitor busy (so the clock ungates) while the first
            # batch's DMAs stream in.
            identity = consts.tile([128, 128], BF16)
            make_identity(nc, identity)
            one1 = consts.tile([1, 1], F32)
            nc.vector.memset(one1, 1.0)
            junk = consts.tile([128, 128], BF16)
            nc.vector.memset(junk, 0.0)
            warm_ps = psum.tile([128, 128], BF16, tag="sc", bufs=1)
            for _ in range(72):
                nc.tensor.transpose(warm_ps, junk, junk)

            # --- replicated weights / biases (sync queue, parallel with
            # the gpsimd encoder stream) --------------------------------
            w1a_sb = consts.tile([128, KC, H], FP8)
            nc.sync.dma_start(out=w1a_sb, in_=w1a8.rearrange("k p h -> p k h"))
            hbe_sb = consts.tile([128, JC, BL], F32)
            nc.sync.dma_start(out=hbe_sb, in_=hbe)
            hbr_sb = consts.tile([128, JC, BL], F32)
            nc.sync.dma_start(out=hbr_sb, in_=hbr)
            w2lp_sb = consts.tile([128, JC, 32], BF16)
            nc.sync.dma_start(out=w2lp_sb, in_=w2lp)
            maskb_sb = consts.tile([128, 2], BF16)
            nc.sync.dma_start(out=maskb_sb, in_=maskb)
            # zero-padded context-weight stationary: zeros come from the
            # host; each epilogue overwrites only column 0 of each block.
            wcol_pad = consts.tile([128, KC, 32], BF16)
            nc.sync.dma_start(out=wcol_pad, in_=wcz)

            def emit_loads(b):
                # gpsimd software-DGE DMAs cast f32 -> bf16 on the fly.
                e_ts = []
                for tt in range(TT):
                    e_t = encp.tile([128, H], BF16, tag="e", bufs=3 * TT,
                                    name=f"e_{b}_{tt}")
                    nc.gpsimd.dma_start(out=e_t, in_=enc[b, tt])
                    e_ts.append(e_t)
                return e_ts

            def alloc_eT(b):
                return etp.tile([128, KC, T], FP8, tag="eT", bufs=2,
                                name=f"eT_{b}")

            def emit_transpose_tt(b, e_ts, eT, tt):
                # all KC h-chunks of one t-subtile -> eT[:, :, tt*128:...]
                # (starts as soon as e_ts[tt]'s DMA lands).  The cast copy
                # alternates DVE/ACT to balance engine load.
                tp = psum.tile([128, KC, 128], BF16, tag="trans", bufs=2)
                for k in range(KC):
                    nc.tensor.transpose(
                        tp[:, k, :],
                        e_ts[tt][:, k * 128:(k + 1) * 128],
                        identity,
                    )
                dst = eT[:, :, tt * 128:(tt + 1) * 128]
                src = tp
                if tt % 2 == 0:
                    nc.vector.tensor_copy(out=dst, in_=src)
                else:
                    nc.scalar.activation(out=dst, in_=src, func=ACTF.Copy,
                                         scale=1.0)

            def emit_scores_tail(b, sc_ps):
                # stage partials in SBUF (bf16), PE-reduce per tg to
                # partition 0, exp with row-sum accumulation.
                scs = scorep.tile([128, 512], BF16, tag="scs")
                nc.scalar.activation(out=scs, in_=sc_ps, func=ACTF.Copy,
                                     scale=1.0)
                expw = scorep.tile([1, T], F32, tag="expw")
                rs2 = scorep.tile([1, 2], F32, tag="rsum2")
                for tg in range(2):
                    scr = psum.tile([1, 512], F32, tag="ctx", bufs=1)
                    nc.tensor.matmul(
                        scr,
                        lhsT=maskb_sb[:, tg:tg + 1],
                        rhs=scs,
                        start=True, stop=True,
                    )
                    nc.scalar.activation(
                        out=expw[:, tg * 512:(tg + 1) * 512], in_=scr,
                        func=ACTF.Exp, scale=1.0, accum_out=rs2[:, tg:tg + 1])
                return expw, rs2

            # ---- epilogue pieces (for batch whose phase1 has finished) --
            def epi_weights(state):
                # weights -> padded column stationary (PE transpose).
                e_ts, expw = state[:2]
                w_ps = psum.tile([128, KC, 1], F32, tag="ctx", bufs=1)
                for c in range(KC):
                    nc.tensor.transpose(
                        w_ps[:, c, :],
                        expw[0:1, c * 128:(c + 1) * 128],
                        one1,
                    )
                nc.vector.tensor_copy(out=wcol_pad[:, :, 0:1], in_=w_ps)

            def epi_context(state):
                # context[h] partials, col-group packed 4-way.
                e_ts = state[0]
                cp = psum.tile([128, 512], F32, tag="ctx", bufs=1)
                for half in range(2):
                    for tch in range(KC):
                        pos = 64 * (tch % 2) + 32 * half
                        nc.tensor.matmul(
                            cp[pos:pos + 32, :],
                            lhsT=wcol_pad[:, tch, :],
                            rhs=e_ts[tch][:, half * 512:(half + 1) * 512],
                            start=(tch < 2),
                            stop=(tch >= KC - 2),
                            tile_position=(0, pos),
                        )
                return cp

            def epi_out(b, cp, state):
                # unnormalized context partials out; the host sums the
                # col-group partial rows and divides by the softmax Z
                # (removes the masked-reduce matmuls + copies from the
                # serial tail).
                rs2 = state[2]
                cps = outp.tile([128, 512], F32, tag="cps")
                nc.scalar.activation(out=cps, in_=cp, func=ACTF.Copy,
                                     scale=1.0)
                nc.sync.dma_start(out=zs[b:b + 1, :], in_=rs2)
                nc.sync.dma_start(out=outp4[b], in_=cps[0:97:32, :])

            def phase1(b, e_ts, eT, next_ctx, prev_ctx, final=False):
                """Main pass for batch b.

                next_ctx: (e_ts, eT) of batch b+1 whose transposes are
                interleaved into this j-loop (or None).
                prev_ctx: (b-1, state) whose epilogue is interleaved
                (or None).
                """
                sc_ps = psum.tile([128, 512], F32, tag="sc", bufs=1)
                s2_prev = None
                epi = {}

                def emit_score(j, s2):
                    for tg in range(2):
                        pos = 64 * (j % 2) + 32 * tg
                        nc.tensor.matmul(
                            sc_ps[pos:pos + 32, :],
                            lhsT=w2lp_sb[:, j, :],
                            rhs=s2[:, tg * 512:(tg + 1) * 512],
                            start=(j < 2),
                            stop=(j >= JC - 2),
                            tile_position=(0, pos),
                        )

                for j in range(JC):
                    mp = psum.tile([128, T], F32, tag="mlp", bufs=2)
                    for tg in range(2):
                        for kk in range(KC // 2):
                            nc.tensor.matmul(
                                mp[:, tg * 512:(tg + 1) * 512],
                                lhsT=w1a_sb[:, 2 * kk:2 * kk + 2,
                                            j * 128:(j + 1) * 128],
                                rhs=eT[:, 2 * kk:2 * kk + 2,
                                       tg * 512:(tg + 1) * 512],
                                start=(kk == 0),
                                stop=(kk == KC // 2 - 1),
                                perf_mode=DR,
                            )
                    if next_ctx is not None:
                        # front-loaded so the last cast-copy lands before
                        # the next batch's first DoubleRow matmul.
                        for tt in ([j] if j < 5 else [5, 6] if j == 5
                                   else [7] if j == 6 else []):
                            emit_transpose_tt(b + 1, next_ctx[0],
                                              next_ctx[1], tt)
                    if prev_ctx is not None:
                        pb, pstate = prev_ctx
                        if j == 4:
                            epi_weights(pstate)
                        elif j == 5:
                            epi["cp"] = epi_context(pstate)
                        elif j == 6:
                            epi_out(pb, epi["cp"], pstate)
                    if s2_prev is not None:
                        emit_score(j - 1, s2_prev)
                    e2 = selup.tile([128, T], BF16, tag="e2")
                    nc.scalar.activation(out=e2, in_=mp, func=ACTF.Exp,
                                         bias=hbe_sb[:, j, b:b + 1],
                                         scale=1.0 / SW)
                    r2 = selup.tile([128, T], BF16, tag="r2")
                    if j % 2 == 0:
                        # ACT path: r2 = relu(mp/SW + hb)
                        nc.scalar.activation(out=r2, in_=mp, func=ACTF.Relu,
                                             bias=hbr_sb[:, j, b:b + 1],
                                             scale=1.0 / SW)
                        alpha_cap = SELU_ALPHA
                    else:
                        # DVE path, SW-scaled: r2 = max(mp + SW*hb, 0);
                        # this j's whole s2 is scaled by SW (host divides
                        # the W2 column by SW and offsets the exp bias).
                        nc.vector.tensor_scalar(
                            out=r2, in0=mp, scalar1=hbr_sb[:, j, b:b + 1],
                            scalar2=0.0, op0=ALU.add, op1=ALU.max,
                        )
                        alpha_cap = SELU_ALPHA * SW
                    # s2 = min(e2, alpha) + r2, single fused DVE op
                    s2 = selup.tile([128, T], BF16, tag="s2", bufs=4)
                    nc.vector.scalar_tensor_tensor(
                        out=s2, in0=e2, scalar=alpha_cap, in1=r2,
                        op0=ALU.min, op1=ALU.add,
                    )
                    s2_prev = s2
                emit_score(JC - 1, s2_prev)
                if final:
                    # stage the partials; the pipelined tail does the rest
                    nonlocal last_scs
                    last_scs = scorep.tile([128, 512], BF16, tag="scs")
                    nc.scalar.activation(out=last_scs, in_=sc_ps,
                                         func=ACTF.Copy, scale=1.0)
                    expw = scorep.tile([1, T], F32, tag="expw")
                    rs2 = scorep.tile([1, 2], F32, tag="rsum2")
                    return (e_ts, expw, rs2)
                expw, rs2 = emit_scores_tail(b, sc_ps)
                return (e_ts, expw, rs2)

            def selu_chain(b, j, mp, width):
                """e2/r2/s2 for one (j, tg-or-full) mp tile; returns s2."""
                e2 = selup.tile([128, width], BF16, tag="e2")
                nc.scalar.activation(out=e2, in_=mp, func=ACTF.Exp,
                                     bias=hbe_sb[:, j, b:b + 1],
                                     scale=1.0 / SW)
                r2 = selup.tile([128, width], BF16, tag="r2")
                if j % 2 == 0:
                    nc.scalar.activation(out=r2, in_=mp, func=ACTF.Relu,
                                         bias=hbr_sb[:, j, b:b + 1],
                                         scale=1.0 / SW)
                    alpha_cap = SELU_ALPHA
                else:
                    nc.vector.tensor_scalar(
                        out=r2, in0=mp, scalar1=hbr_sb[:, j, b:b + 1],
                        scalar2=0.0, op0=ALU.add, op1=ALU.max,
                    )
                    alpha_cap = SELU_ALPHA * SW
                s2 = selup.tile([128, width], BF16, tag="s2", bufs=4)
                nc.vector.scalar_tensor_tensor(
                    out=s2, in0=e2, scalar=alpha_cap, in1=r2,
                    op0=ALU.min, op1=ALU.add,
                )
                return s2

            def phase1_split(b, e_ts, eT, next_ctx, prev_ctx, own_tail):
                """Per-t-group pass for the first / last batch.

                b==0: tg0's j-loop only needs transpose groups tt0-3, so it
                starts ~4 DMAs earlier; own tt4-7 and batch 1's transposes
                are interleaved.  b==BL-1: tg0's softmax / weight transpose
                / first context half overlap tg1's j-loop so the serial
                tail shrinks.  Scores here accumulate sequentially into a
                [1,512] row per tg (no staging/reduce hop).
                """
                expw = scorep.tile([1, T], F32, tag="expw")
                rs2 = scorep.tile([1, 2], F32, tag="rsum2")
                epi = {}

                for tg in range(2):
                    sc_row = psum.tile([1, 512], F32, tag="sc", bufs=1)
                    tgs = slice(tg * 512, (tg + 1) * 512)
                    s2q = []

                    def emit_score_seq(j, s2):
                        nc.tensor.matmul(
                            sc_row,
                            lhsT=w2lp_sb[:, j, 0:1],
                            rhs=s2,
                            start=(j == 0),
                            stop=(j == JC - 1),
                        )

                    for j in range(JC):
                        mp = psum.tile([128, 512], F32, tag="mlp", bufs=2)
                        for kk in range(KC // 2):
                            nc.tensor.matmul(
                                mp,
                                lhsT=w1a_sb[:, 2 * kk:2 * kk + 2,
                                            j * 128:(j + 1) * 128],
                                rhs=eT[:, 2 * kk:2 * kk + 2, tgs],
                                start=(kk == 0),
                                stop=(kk == KC // 2 - 1),
                                perf_mode=DR,
                            )
                        if own_tail and tg == 0 and prev_ctx is not None:
                            pb, pstate = prev_ctx
                            if j == 4:
                                epi_weights(pstate)
                            elif j == 5:
                                epi["pcp"] = epi_context(pstate)
                            elif j == 6:
                                epi_out(pb, epi["pcp"], pstate)
                        if own_tail and tg == 1:
                            if j == 0:
                                # softmax of tg0 while tg1 streams
                                nc.scalar.activation(
                                    out=expw[:, 0:512], in_=epi["sc0"],
                                    func=ACTF.Exp, scale=1.0,
                                    accum_out=rs2[:, 0:1])
                            elif j == 2:
                                wps_a = psum.tile([128, 4, 1], F32,
                                                  tag="ctx", bufs=1)
                                for c in range(4):
                                    nc.tensor.transpose(
                                        wps_a[:, c, :],
                                        expw[0:1, c * 128:(c + 1) * 128],
                                        one1,
                                    )
                                nc.vector.tensor_copy(
                                    out=wcol_pad[:, 0:4, 0:1], in_=wps_a)
                            elif j == 4:
                                cp = psum.tile([128, 512], F32, tag="ctx",
                                               bufs=1)
                                epi["cp"] = cp
                                for half in range(2):
                                    for tch in range(4):
                                        pos = 64 * (tch % 2) + 32 * half
                                        nc.tensor.matmul(
                                            cp[pos:pos + 32, :],
                                            lhsT=wcol_pad[:, tch, :],
                                            rhs=e_ts[tch][:, half * 512:
                                                          (half + 1) * 512],
                                            start=(tch < 2),
                                            stop=False,
                                            tile_position=(0, pos),
                                        )
                        # deep score deferral (j-3): at half-width pacing
                        # the selu chain needs ~3 PE j-steps of slack before
                        # the score matmul consumes its s2
                        if len(s2q) >= 3:
                            emit_score_seq(j - 3, s2q[-3])
                        s2q.append(selu_chain(b, j, mp, 512))
                    for jr in (JC - 3, JC - 2, JC - 1):
                        emit_score_seq(jr, s2q[jr])
                    if tg == 0:
                        epi["sc0"] = sc_row
                        if not own_tail:
                            nc.scalar.activation(
                                out=expw[:, 0:512], in_=sc_row,
                                func=ACTF.Exp, scale=1.0,
                                accum_out=rs2[:, 0:1])
                    else:
                        nc.scalar.activation(
                            out=expw[:, 512:1024], in_=sc_row,
                            func=ACTF.Exp, scale=1.0, accum_out=rs2[:, 1:2])

                if not own_tail:
                    return (e_ts, expw, rs2)

                # remaining tail: weight transposes c4-7, context tch4-7,
                # reduce, out
                wps_b = psum.tile([128, 4, 1], F32, tag="sc", bufs=1)
                for c in range(4):
                    nc.tensor.transpose(
                        wps_b[:, c, :],
                        expw[0:1, (c + 4) * 128:(c + 5) * 128],
                        one1,
                    )
                nc.vector.tensor_copy(out=wcol_pad[:, 4:8, 0:1], in_=wps_b)
                cp = epi["cp"]
                for half in range(2):
                    for tch in range(4, KC):
                        pos = 64 * (tch % 2) + 32 * half
                        nc.tensor.matmul(
                            cp[pos:pos + 32, :],
                            lhsT=wcol_pad[:, tch, :],
                            rhs=e_ts[tch][:, half * 512:(half + 1) * 512],
                            start=False,
                            stop=(tch >= KC - 2),
                            tile_position=(0, pos),
                        )
                epi_out(b, cp, (e_ts, expw, rs2))
                return None

            # ---------------- top-level software pipeline ----------------
            e_ts = emit_loads(0)
            eT = alloc_eT(0)
            for tt in range(TT):
                emit_transpose_tt(0, e_ts, eT, tt)

            prev_state = None
            for b in range(0, BL - 1):
                e_ts_n = emit_loads(b + 1)
                next_ctx = (e_ts_n, alloc_eT(b + 1))
                prev_ctx = (b - 1, prev_state) if prev_state is not None \
                    else None
                state = phase1(b, e_ts, eT, next_ctx, prev_ctx)
                prev_state = state
                e_ts, eT = next_ctx

            last_scs = None
            state = phase1(BL - 1, e_ts, eT, None, (BL - 2, prev_state),
                           final=True)
            # pipelined final tail: the second reduce rides the freed sc
            # bank so both reduces issue back-to-back, and each half's
            # weight transposes run between the two exps.
            e_ts_l, expw_l, rs2_l = state
            scr0 = psum.tile([1, 512], F32, tag="ctx", bufs=1)
            nc.tensor.matmul(scr0, lhsT=maskb_sb[:, 0:1], rhs=last_scs,
                             start=True, stop=True)
            scr1 = psum.tile([1, 512], F32, tag="sc", bufs=1)
            nc.tensor.matmul(scr1, lhsT=maskb_sb[:, 1:2], rhs=last_scs,
                             start=True, stop=True)
            nc.scalar.activation(out=expw_l[:, 0:512], in_=scr0,
                                 func=ACTF.Exp, scale=1.0,
                                 accum_out=rs2_l[:, 0:1])
            wps_a = psum.tile([128, 4, 1], F32, tag="ctx", bufs=1)
            for c in range(4):
                nc.tensor.transpose(wps_a[:, c, :],
                                    expw_l[0:1, c * 128:(c + 1) * 128], one1)
            nc.vector.tensor_copy(out=wcol_pad[:, 0:4, 0:1], in_=wps_a)
            nc.scalar.activation(out=expw_l[:, 512:1024], in_=scr1,
                                 func=ACTF.Exp, scale=1.0,
                                 accum_out=rs2_l[:, 1:2])
            wps_b = psum.tile([128, 4, 1], F32, tag="sc", bufs=1)
            for c in range(4):
                nc.tensor.transpose(
                    wps_b[:, c, :],
                    expw_l[0:1, (c + 4) * 128:(c + 5) * 128], one1)
            nc.vector.tensor_copy(out=wcol_pad[:, 4:8, 0:1], in_=wps_b)
            cp = epi_context(state)
            epi_out(BL - 1, cp, state)

    nc.compile()
    return nc


_NC_CACHE = None


def _get_nc():
    global _NC_CACHE
    if _NC_CACHE is None:
        _NC_CACHE = build_kernel()
    return _NC_CACHE


def make_in_maps(encoder_outputs, hidden_state, W1, b1, W2):
    enc = np.ascontiguousarray(np.asarray(encoder_outputs, np.float32))
    hid = np.ascontiguousarray(np.asarray(hidden_state, np.float32))
    W1 = np.asarray(W1, np.float32)
    b1 = np.asarray(b1, np.float32)
    W2 = np.asarray(W2, np.float32)

    bf16 = ml_dtypes.bfloat16
    f8 = ml_dtypes.float8_e4m3
    # cast to the HW e4m3 format, but ship the bytes under the e4m3fn
    # container dtype: the PJRT path rejects the IEEE f8E4M3 HLO type
    # while accepting f8E4M3FN, and bass's input check is fuzzy across
    # the two.
    w1a8 = np.ascontiguousarray(
        (W1[:H] * SW).reshape(KC, 128, H)).astype(f8).view(
            ml_dtypes.float8_e4m3fn)

    # per-j s2 scale: odd j's selu output is scaled by SW (DVE relu path)
    jscale = np.where(np.arange(JC) % 2 == 1, SW, 1.0).astype(np.float32)
    w2l = (W2[:, 0] * SELU_LAMBDA).reshape(JC, 128) / jscale[:, None]
    w2lp = np.zeros((128, JC, 32), bf16)
    w2lp[:, :, 0] = w2l.T.astype(bf16)

    # reduction masks: tg0/half0 partials live at partitions [0,32) and
    # [64,96) (real rows 0 and 64, zeros elsewhere), tg1/half1 at the
    # complement.
    m = np.zeros((128, 2), np.float32)
    m[0:32, 0] = 1.0
    m[64:96, 0] = 1.0
    m[32:64, 1] = 1.0
    m[96:128, 1] = 1.0

    # host-side hidden-state contribution: hb[b, :] = hid[b] @ W1[H:] + b1
    hb_all = hid[0] @ W1[H:] + b1                       # (B, H) f32
    ln_alpha = math.log(SELU_ALPHA)
    ln_sw = math.log(SW)

    in_maps = []
    for c in range(N_CORES):
        sl = slice(BL * c, BL * (c + 1))
        hb = hb_all[sl].reshape(BL, JC, 128).transpose(2, 1, 0)  # (128,JC,BL)
        # exp bias: hb + ln(alpha) (+ ln(SW) for odd j so e2 = SW*alpha*e^x)
        hbe = hb + ln_alpha + ln_sw * (np.arange(JC) % 2)[None, :, None]
        # relu bias: hb (ACT, even j) or SW*hb (DVE, odd j)
        hbr = hb * np.where(np.arange(JC) % 2 == 1, SW, 1.0)[None, :, None]
        in_maps.append({
            "enc": np.ascontiguousarray(enc[sl]).reshape(BL, TT, 128, H),
            "w1a8": w1a8,
            "w2lp": w2lp,
            "hbe": np.ascontiguousarray(hbe.astype(np.float32)),
            "hbr": np.ascontiguousarray(hbr.astype(np.float32)),
            "maskb": m.astype(bf16),
            "wcz": np.zeros((128, KC, 32), bf16),
        })
    return in_maps


def kernel(encoder_outputs, hidden_state, W1, b1, W2, b2):
    # b2 shifts every score equally; softmax is shift-invariant, so it is
    # deliberately unused.
    in_maps = make_in_maps(encoder_outputs, hidden_state, W1, b1, W2)
    nc = _get_nc()
    res = run_bass_kernel_spmd(nc, in_maps, core_ids=list(range(N_CORES)))
    out = np.empty((1, B, H), np.float32)
    for c in range(N_CORES):
        z = res.results[c]["zs"].sum(axis=1, keepdims=True)   # (BL, 1)
        p4 = res.results[c]["outp4"]                          # (BL, 4, 512)
        ctx = np.concatenate([p4[:, 0] + p4[:, 2],
                              p4[:, 1] + p4[:, 3]], axis=1)   # (BL, H)
        out[0, BL * c:BL * (c + 1)] = ctx / z
    return out


# revision 41
# speedup vs baseline: 1.1405x; 1.1405x over previous
"""Additive (Bahdanau-style) attention kernel for Trainium2, 8 NeuronCores.

reference computation (per batch b of 32, T=1024 timesteps, H=1024):
    mlp_hidden = selu([enc[b,t]; hid[b]] @ W1 + b1)     # (T, H)
    scores     = mlp_hidden @ W2 + b2                   # (T, 1)
    weights    = softmax(scores over t)
    out[b]     = sum_t weights[t] * enc[b,t]            # (H,)

Distribution: data-parallel over batch, 4 batches per core, no collectives.

Per-core algorithm (v3, fp8 DoubleRow, software-pipelined):
  - the hid @ W1[H:] + b1 term is per-batch constant; precomputed on the
    host and shipped as per-(j,b) bias columns (relu / exp variants).
  - scores path in fp8e4: E loaded once from HBM as bf16 (DMA cast),
    PE-transposed per 128x128 block, PSUM->SBUF copy casts to fp8.  W1a
    host-cast to fp8 with a 2^10 scale (entries ~1e-2 sit below fp8's
    normal range); the descale rides the ACT/DVE affine inputs.  The mlp
    matmul uses perf_mode=DoubleRow: one instruction contracts 256 rows.
  - selu = max(x,0) + min(alpha*e^x, alpha) (+const, dropped by softmax).
    exp always on ACT; the relu alternates between ACT (even j) and DVE
    tensor_scalar (odd j) to keep the j-loop from being ACT-bound.  For
    DVE j's the whole s2 is scaled by SW (host pre-scales that j's bias
    by SW and W2 column by 1/SW) so no extra scaling op is needed.
  - eT cast-copies alternate DVE / ACT per k for the same reason.
  - scores dot / context matmul have 1-wide outputs; packed 4-way into
    disjoint 32-column PE groups via tile_position (stationaries zero-
    padded to 32 cols so all PSUM partitions are written), then reduced
    across the 4 partial rows by a masked-ones PE matmul to partition 0.
  - software pipeline per batch b: loads(b+1) up front; transpose group
    tt of batch b+1 emitted after main-matmul group j=tt; epilogue(b-1)
    pieces emitted after j=4/5/6 so its matmuls land mid-stream (PE warm,
    no HAM re-throttle); score tail at the end of the j-loop.
  - softmax: exp off the reduced score rows (scores O(1), no max shift);
    1/Z folded into the output copy's scale.
"""

import math

import ml_dtypes
import numpy as np

import concourse.tile as tile
from concourse.masks import make_identity
from concourse import bacc, mybir
from concourse.bass_utils import run_bass_kernel_spmd

F32 = mybir.dt.float32
BF16 = mybir.dt.bfloat16
FP8 = mybir.dt.float8e4
ALU = mybir.AluOpType
ACTF = mybir.ActivationFunctionType
DR = mybir.MatmulPerfMode.DoubleRow

N_CORES = 8
B = 32
T = 1024
H = 1024
BL = B // N_CORES          # batches per core = 4
KC = H // 128              # contraction chunks = 8
JC = H // 128              # hidden-unit chunks = 8
TT = T // 128              # 128-row t-subtiles per batch = 8

SELU_LAMBDA = 1.0507009873554805
SELU_ALPHA = 1.6732632423543772
SW = 1024.0                # fp8 scale for W1a (and odd-j s2 scale)


def build_kernel():
    nc = bacc.Bacc("TRN2", target_bir_lowering=False, debug=False,
                   num_devices=N_CORES)

    enc = nc.dram_tensor("enc", [BL, TT, 128, H], F32, kind="ExternalInput").ap()
    w1a8 = nc.dram_tensor("w1a8", [KC, 128, H], FP8, kind="ExternalInput").ap()
    w2lp = nc.dram_tensor("w2lp", [128, JC, 32], BF16, kind="ExternalInput").ap()
    hbe = nc.dram_tensor("hbe", [128, JC, BL], F32, kind="ExternalInput").ap()
    hbr = nc.dram_tensor("hbr", [128, JC, BL], F32, kind="ExternalInput").ap()
    maskb = nc.dram_tensor("maskb", [128, 2], BF16, kind="ExternalInput").ap()
    wcz = nc.dram_tensor("wcz", [128, KC, 32], BF16, kind="ExternalInput").ap()
    outp4 = nc.dram_tensor("outp4", [BL, 4, 512], F32,
                           kind="ExternalOutput").ap()
    zs = nc.dram_tensor("zs", [BL, 2], F32, kind="ExternalOutput").ap()

    with tile.TileContext(nc) as tc:
        with (
            tc.tile_pool(name="consts", bufs=1) as consts,
            tc.tile_pool(name="encp", bufs=3) as encp,
            tc.tile_pool(name="etp", bufs=2) as etp,
            tc.tile_pool(name="selu", bufs=4) as selup,
            tc.tile_pool(name="score", bufs=2) as scorep,
            tc.tile_pool(name="outp", bufs=2) as outp,
            tc.tile_pool(name="psum", bufs=2, space="PSUM") as psum,
        ):
            # identity + PE warmup first: the warmup keeps the TensorE
            # activity monitor busy (so the clock ungates) while the first
            # batch's DMAs stream in.
            identity = consts.tile([128, 128], BF16)
            make_identity(nc, identity)
            one1 = consts.tile([1, 1], F32)
            nc.vector.memset(one1, 1.0)
            junk = consts.tile([128, 128], BF16)
            nc.vector.memset(junk, 0.0)
            warm_ps = psum.tile([128, 128], BF16, tag="sc", bufs=1)
            for _ in range(72):
                nc.tensor.transpose(warm_ps, junk, junk)

            # --- replicated weights / biases (sync queue, parallel with
            # the gpsimd encoder stream) --------------------------------
            w1a_sb = consts.tile([128, KC, H], FP8)
            nc.sync.dma_start(out=w1a_sb, in_=w1a8.rearrange("k p h -> p k h"))
            hbe_sb = consts.tile([128, JC, BL], F32)
            nc.sync.dma_start(out=hbe_sb, in_=hbe)
            hbr_sb = consts.tile([128, JC, BL], F32)
            nc.sync.dma_start(out=hbr_sb, in_=hbr)
            w2lp_sb = consts.tile([128, JC, 32], BF16)
            nc.sync.dma_start(out=w2lp_sb, in_=w2lp)
            maskb_sb = consts.tile([128, 2], BF16)
            nc.sync.dma_start(out=maskb_sb, in_=maskb)
            # zero-padded context-weight stationary: zeros come from the
            # host; each epilogue overwrites only column 0 of each block.
            wcol_pad = consts.tile([128, KC, 32], BF16)
            nc.sync.dma_start(out=wcol_pad, in_=wcz)

            def emit_loads(b):
                # gpsimd software-DGE DMAs cast f32 -> bf16 on the fly.
                e_ts = []
                for tt in range(TT):
                    e_t = encp.tile([128, H], BF16, tag="e", bufs=3 * TT,
                                    name=f"e_{b}_{tt}")
                    nc.gpsimd.dma_start(out=e_t, in_=enc[b, tt])
                    e_ts.append(e_t)
                return e_ts

            def alloc_eT(b):
                return etp.tile([128, KC, T], FP8, tag="eT", bufs=2,
                                name=f"eT_{b}")

            def emit_transpose_tt(b, e_ts, eT, tt):
                # all KC h-chunks of one t-subtile -> eT[:, :, tt*128:...]
                # (starts as soon as e_ts[tt]'s DMA lands).  The cast copy
                # alternates DVE/ACT to balance engine load.
                tp = psum.tile([128, KC, 128], BF16, tag="trans", bufs=2)
                for k in range(KC):
                    nc.tensor.transpose(
                        tp[:, k, :],
                        e_ts[tt][:, k * 128:(k + 1) * 128],
                        identity,
                    )
                dst = eT[:, :, tt * 128:(tt + 1) * 128]
                src = tp
                if tt in (2, 4, 6, 7):
                    nc.vector.tensor_copy(out=dst, in_=src)
                else:
                    nc.scalar.activation(out=dst, in_=src, func=ACTF.Copy,
                                         scale=1.0)

            def emit_scores_tail(b, sc_ps):
                # stage partials in SBUF (bf16), PE-reduce per tg to
                # partition 0, exp with row-sum accumulation.
                scs = scorep.tile([128, 512], BF16, tag="scs")
                nc.scalar.activation(out=scs, in_=sc_ps, func=ACTF.Copy,
                                     scale=1.0)
                expw = scorep.tile([1, T], F32, tag="expw")
                rs2 = scorep.tile([1, 2], F32, tag="rsum2")
                for tg in range(2):
                    scr = psum.tile([1, 512], F32, tag="ctx", bufs=1)
                    nc.tensor.matmul(
                        scr,
                        lhsT=maskb_sb[:, tg:tg + 1],
                        rhs=scs,
                        start=True, stop=True,
                    )
                    nc.scalar.activation(
                        out=expw[:, tg * 512:(tg + 1) * 512], in_=scr,
                        func=ACTF.Exp, scale=1.0, accum_out=rs2[:, tg:tg + 1])
                return expw, rs2

            # ---- epilogue pieces (for batch whose phase1 has finished) --
            def epi_weights(state):
                # weights -> padded column stationary (PE transpose).
                e_ts, expw = state[:2]
                w_ps = psum.tile([128, KC, 1], F32, tag="ctx", bufs=1)
                for c in range(KC):
                    nc.tensor.transpose(
                        w_ps[:, c, :],
                        expw[0:1, c * 128:(c + 1) * 128],
                        one1,
                    )
                nc.vector.tensor_copy(out=wcol_pad[:, :, 0:1], in_=w_ps)

            def epi_context(state):
                # context[h] partials, col-group packed 4-way.
                e_ts = state[0]
                cp = psum.tile([128, 512], F32, tag="ctx", bufs=1)
                for half in range(2):
                    for tch in range(KC):
                        pos = 64 * (tch % 2) + 32 * half
                        nc.tensor.matmul(
                            cp[pos:pos + 32, :],
                            lhsT=wcol_pad[:, tch, :],
                            rhs=e_ts[tch][:, half * 512:(half + 1) * 512],
                            start=(tch < 2),
                            stop=(tch >= KC - 2),
                            tile_position=(0, pos),
                        )
                return cp

            def epi_out(b, cp, state):
                # unnormalized context partials out; the host sums the
                # col-group partial rows and divides by the softmax Z
                # (removes the masked-reduce matmuls + copies from the
                # serial tail).
                rs2 = state[2]
                cps = outp.tile([128, 512], F32, tag="cps")
                nc.scalar.activation(out=cps, in_=cp, func=ACTF.Copy,
                                     scale=1.0)
                nc.sync.dma_start(out=zs[b:b + 1, :], in_=rs2)
                nc.sync.dma_start(out=outp4[b], in_=cps[0:97:32, :])

            def phase1(b, e_ts, eT, next_ctx, prev_ctx, final=False):
                """Main pass for batch b.

                next_ctx: (e_ts, eT) of batch b+1 whose transposes are
                interleaved into this j-loop (or None).
                prev_ctx: (b-1, state) whose epilogue is interleaved
                (or None).
                """
                sc_ps = psum.tile([128, 512], F32, tag="sc", bufs=1)
                s2_prev = None
                epi = {}

                def emit_score(j, s2):
                    for tg in range(2):
                        pos = 64 * (j % 2) + 32 * tg
                        nc.tensor.matmul(
                            sc_ps[pos:pos + 32, :],
                            lhsT=w2lp_sb[:, j, :],
                            rhs=s2[:, tg * 512:(tg + 1) * 512],
                            start=(j < 2),
                            stop=(j >= JC - 2),
                            tile_position=(0, pos),
                        )

                for j in range(JC):
                    mp = psum.tile([128, T], F32, tag="mlp", bufs=2)
                    for tg in range(2):
                        for kk in range(KC // 2):
                            nc.tensor.matmul(
                                mp[:, tg * 512:(tg + 1) * 512],
                                lhsT=w1a_sb[:, 2 * kk:2 * kk + 2,
                                            j * 128:(j + 1) * 128],
                                rhs=eT[:, 2 * kk:2 * kk + 2,
                                       tg * 512:(tg + 1) * 512],
                                start=(kk == 0),
                                stop=(kk == KC // 2 - 1),
                                perf_mode=DR,
                            )
                    if next_ctx is not None:
                        # front-loaded so the last cast-copy lands before
                        # the next batch's first DoubleRow matmul.
                        for tt in ([j] if j < 5 else [5, 6] if j == 5
                                   else [7] if j == 6 else []):
                            emit_transpose_tt(b + 1, next_ctx[0],
                                              next_ctx[1], tt)
                    if prev_ctx is not None:
                        pb, pstate = prev_ctx
                        if j == 4:
                            epi_weights(pstate)
                        elif j == 5:
                            epi["cp"] = epi_context(pstate)
                        elif j == 6:
                            epi_out(pb, epi["cp"], pstate)
                    if s2_prev is not None:
                        emit_score(j - 1, s2_prev)
                    e2 = selup.tile([128, T], BF16, tag="e2")
                    nc.scalar.activation(out=e2, in_=mp, func=ACTF.Exp,
                                         bias=hbe_sb[:, j, b:b + 1],
                                         scale=1.0 / SW)
                    r2 = selup.tile([128, T], BF16, tag="r2")
                    if j % 2 == 0:
                        # ACT path: r2 = relu(mp/SW + hb)
                        nc.scalar.activation(out=r2, in_=mp, func=ACTF.Relu,
                                             bias=hbr_sb[:, j, b:b + 1],
                                             scale=1.0 / SW)
                        alpha_cap = SELU_ALPHA
                    else:
                        # DVE path, SW-scaled: r2 = max(mp + SW*hb, 0);
                        # this j's whole s2 is scaled by SW (host divides
                        # the W2 column by SW and offsets the exp bias).
                        nc.vector.tensor_scalar(
                            out=r2, in0=mp, scalar1=hbr_sb[:, j, b:b + 1],
                            scalar2=0.0, op0=ALU.add, op1=ALU.max,
                        )
                        alpha_cap = SELU_ALPHA * SW
                    # s2 = min(e2, alpha) + r2, single fused DVE op
                    s2 = selup.tile([128, T], BF16, tag="s2", bufs=4)
                    nc.vector.scalar_tensor_tensor(
                        out=s2, in0=e2, scalar=alpha_cap, in1=r2,
                        op0=ALU.min, op1=ALU.add,
                    )
                    s2_prev = s2
                emit_score(JC - 1, s2_prev)
                if final:
                    # stage the partials; the pipelined tail does the rest
                    nonlocal last_scs
                    last_scs = scorep.tile([128, 512], BF16, tag="scs")
                    nc.scalar.activation(out=last_scs, in_=sc_ps,
                                         func=ACTF.Copy, scale=1.0)
                    expw = scorep.tile([1, T], F32, tag="expw")
                    rs2 = scorep.tile([1, 2], F32, tag="rsum2")
                    return (e_ts, expw, rs2)
                expw, rs2 = emit_scores_tail(b, sc_ps)
                return (e_ts, expw, rs2)

            def selu_chain(b, j, mp, width):
                """e2/r2/s2 for one (j, tg-or-full) mp tile; returns s2."""
                e2 = selup.tile([128, width], BF16, tag="e2")
                nc.scalar.activation(out=e2, in_=mp, func=ACTF.Exp,
                                     bias=hbe_sb[:, j, b:b + 1],
                                     scale=1.0 / SW)
                r2 = selup.tile([128, width], BF16, tag="r2")
                if j % 2 == 0:
                    nc.scalar.activation(out=r2, in_=mp, func=ACTF.Relu,
                                         bias=hbr_sb[:, j, b:b + 1],
                                         scale=1.0 / SW)
                    alpha_cap = SELU_ALPHA
                else:
                    nc.vector.tensor_scalar(
                        out=r2, in0=mp, scalar1=hbr_sb[:, j, b:b + 1],
                        scalar2=0.0, op0=ALU.add, op1=ALU.max,
                    )
                    alpha_cap = SELU_ALPHA * SW
                s2 = selup.tile([128, width], BF16, tag="s2", bufs=4)
                nc.vector.scalar_tensor_tensor(
                    out=s2, in0=e2, scalar=alpha_cap, in1=r2,
                    op0=ALU.min, op1=ALU.add,
                )
                return s2

            def phase1_split(b, e_ts, eT, next_ctx, prev_ctx, own_tail):
                """Per-t-group pass for the first / last batch.

                b==0: tg0's j-loop only needs transpose groups tt0-3, so it
                starts ~4 DMAs earlier; own tt4-7 and batch 1's transposes
                are interleaved.  b==BL-1: tg0's softmax / weight transpose
                / first context half overlap tg1's j-loop so the serial
                tail shrinks.  Scores here accumulate sequentially into a
                [1,512] row per tg (no staging/reduce hop).
                """
                expw = scorep.tile([1, T], F32, tag="expw")
                rs2 = scorep.tile([1, 2], F32, tag="rsum2")
                epi = {}

                for tg in range(2):
                    sc_row = psum.tile([1, 512], F32, tag="sc", bufs=1)
                    tgs = slice(tg * 512, (tg + 1) * 512)
                    s2q = []

                    def emit_score_seq(j, s2):
                        nc.tensor.matmul(
                            sc_row,
                            lhsT=w2lp_sb[:, j, 0:1],
                            rhs=s2,
                            start=(j == 0),
                            stop=(j == JC - 1),
                        )

                    for j in range(JC):
                        mp = psum.tile([128, 512], F32, tag="mlp", bufs=2)
                        for kk in range(KC // 2):
                            nc.tensor.matmul(
                                mp,
                                lhsT=w1a_sb[:, 2 * kk:2 * kk + 2,
                                            j * 128:(j + 1) * 128],
                                rhs=eT[:, 2 * kk:2 * kk + 2, tgs],
                                start=(kk == 0),
                                stop=(kk == KC // 2 - 1),
                                perf_mode=DR,
                            )
                        if own_tail and tg == 0 and prev_ctx is not None:
                            pb, pstate = prev_ctx
                            if j == 4:
                                epi_weights(pstate)
                            elif j == 5:
                                epi["pcp"] = epi_context(pstate)
                            elif j == 6:
                                epi_out(pb, epi["pcp"], pstate)
                        if own_tail and tg == 1:
                            if j == 0:
                                # softmax of tg0 while tg1 streams
                                nc.scalar.activation(
                                    out=expw[:, 0:512], in_=epi["sc0"],
                                    func=ACTF.Exp, scale=1.0,
                                    accum_out=rs2[:, 0:1])
                            elif j == 2:
                                wps_a = psum.tile([128, 4, 1], F32,
                                                  tag="ctx", bufs=1)
                                for c in range(4):
                                    nc.tensor.transpose(
                                        wps_a[:, c, :],
                                        expw[0:1, c * 128:(c + 1) * 128],
                                        one1,
                                    )
                                nc.vector.tensor_copy(
                                    out=wcol_pad[:, 0:4, 0:1], in_=wps_a)
                            elif j == 4:
                                cp = psum.tile([128, 512], F32, tag="ctx",
                                               bufs=1)
                                epi["cp"] = cp
                                for half in range(2):
                                    for tch in range(4):
                                        pos = 64 * (tch % 2) + 32 * half
                                        nc.tensor.matmul(
                                            cp[pos:pos + 32, :],
                                            lhsT=wcol_pad[:, tch, :],
                                            rhs=e_ts[tch][:, half * 512:
                                                          (half + 1) * 512],
                                            start=(tch < 2),
                                            stop=False,
                                            tile_position=(0, pos),
                                        )
                        # deep score deferral (j-3): at half-width pacing
                        # the selu chain needs ~3 PE j-steps of slack before
                        # the score matmul consumes its s2
                        if len(s2q) >= 3:
                            emit_score_seq(j - 3, s2q[-3])
                        s2q.append(selu_chain(b, j, mp, 512))
                    for jr in (JC - 3, JC - 2, JC - 1):
                        emit_score_seq(jr, s2q[jr])
                    if tg == 0:
                        epi["sc0"] = sc_row
                        if not own_tail:
                            nc.scalar.activation(
                                out=expw[:, 0:512], in_=sc_row,
                                func=ACTF.Exp, scale=1.0,
                                accum_out=rs2[:, 0:1])
                    else:
                        nc.scalar.activation(
                            out=expw[:, 512:1024], in_=sc_row,
                            func=ACTF.Exp, scale=1.0, accum_out=rs2[:, 1:2])

                if not own_tail:
                    return (e_ts, expw, rs2)

                # remaining tail: weight transposes c4-7, context tch4-7,
                # reduce, out
                wps_b = psum.tile([128, 4, 1], F32, tag="sc", bufs=1)
                for c in range(4):
                    nc.tensor.transpose(
                        wps_b[:, c, :],
                        expw[0:1, (c + 4) * 128:(c + 5) * 128],
                        one1,
                    )
                nc.vector.tensor_copy(out=wcol_pad[:, 4:8, 0:1], in_=wps_b)
                cp = epi["cp"]
                for half in range(2):
                    for tch in range(4, KC):
                        pos = 64 * (tch % 2) + 32 * half
                        nc.tensor.matmul(
                            cp[pos:pos + 32, :],
                            lhsT=wcol_pad[:, tch, :],
                            rhs=e_ts[tch][:, half * 512:(half + 1) * 512],
                            start=False,
                            stop=(tch >= KC - 2),
                            tile_position=(0, pos),
                        )
                epi_out(b, cp, (e_ts, expw, rs2))
                return None

            # ---------------- top-level software pipeline ----------------
            e_ts = emit_loads(0)
            eT = alloc_eT(0)
            for tt in range(TT):
                emit_transpose_tt(0, e_ts, eT, tt)

            prev_state = None
            for b in range(0, BL - 1):
                e_ts_n = emit_loads(b + 1)
                next_ctx = (e_ts_n, alloc_eT(b + 1))
                prev_ctx = (b - 1, prev_state) if prev_state is not None \
                    else None
                state = phase1(b, e_ts, eT, next_ctx, prev_ctx)
                prev_state = state
                e_ts, eT = next_ctx

            last_scs = None
            state = phase1(BL - 1, e_ts, eT, None, (BL - 2, prev_state),
                           final=True)
            # pipelined final tail: the second reduce rides the freed sc
            # bank so both reduces issue back-to-back, and each half's
            # weight transposes run between the two exps.
            e_ts_l, expw_l, rs2_l = state
            scr0 = psum.tile([1, 512], F32, tag="ctx", bufs=1)
            nc.tensor.matmul(scr0, lhsT=maskb_sb[:, 0:1], rhs=last_scs,
                             start=True, stop=True)
            scr1 = psum.tile([1, 512], F32, tag="sc", bufs=1)
            nc.tensor.matmul(scr1, lhsT=maskb_sb[:, 1:2], rhs=last_scs,
                             start=True, stop=True)
            nc.scalar.activation(out=expw_l[:, 0:512], in_=scr0,
                                 func=ACTF.Exp, scale=1.0,
                                 accum_out=rs2_l[:, 0:1])
            wps_a = psum.tile([128, 4, 1], F32, tag="ctx", bufs=1)
            for c in range(4):
                nc.tensor.transpose(wps_a[:, c, :],
                                    expw_l[0:1, c * 128:(c + 1) * 128], one1)
            nc.vector.tensor_copy(out=wcol_pad[:, 0:4, 0:1], in_=wps_a)
            nc.scalar.activation(out=expw_l[:, 512:1024], in_=scr1,
                                 func=ACTF.Exp, scale=1.0,
                                 accum_out=rs2_l[:, 1:2])
            wps_b = psum.tile([128, 4, 1], F32, tag="sc", bufs=1)
            for c in range(4):
                nc.tensor.transpose(
                    wps_b[:, c, :],
                    expw_l[0:1, (c + 4) * 128:(c + 5) * 128], one1)
            nc.vector.tensor_copy(out=wcol_pad[:, 4:8, 0:1], in_=wps_b)
            cp = epi_context(state)
            epi_out(BL - 1, cp, state)

    nc.compile()
    return nc


_NC_CACHE = None


def _get_nc():
    global _NC_CACHE
    if _NC_CACHE is None:
        _NC_CACHE = build_kernel()
    return _NC_CACHE


def make_in_maps(encoder_outputs, hidden_state, W1, b1, W2):
    enc = np.ascontiguousarray(np.asarray(encoder_outputs, np.float32))
    hid = np.ascontiguousarray(np.asarray(hidden_state, np.float32))
    W1 = np.asarray(W1, np.float32)
    b1 = np.asarray(b1, np.float32)
    W2 = np.asarray(W2, np.float32)

    bf16 = ml_dtypes.bfloat16
    f8 = ml_dtypes.float8_e4m3
    # cast to the HW e4m3 format, but ship the bytes under the e4m3fn
    # container dtype: the PJRT path rejects the IEEE f8E4M3 HLO type
    # while accepting f8E4M3FN, and bass's input check is fuzzy across
    # the two.
    w1a8 = np.ascontiguousarray(
        (W1[:H] * SW).reshape(KC, 128, H)).astype(f8).view(
            ml_dtypes.float8_e4m3fn)

    # per-j s2 scale: odd j's selu output is scaled by SW (DVE relu path)
    jscale = np.where(np.arange(JC) % 2 == 1, SW, 1.0).astype(np.float32)
    w2l = (W2[:, 0] * SELU_LAMBDA).reshape(JC, 128) / jscale[:, None]
    w2lp = np.zeros((128, JC, 32), bf16)
    w2lp[:, :, 0] = w2l.T.astype(bf16)

    # reduction masks: tg0/half0 partials live at partitions [0,32) and
    # [64,96) (real rows 0 and 64, zeros elsewhere), tg1/half1 at the
    # complement.
    m = np.zeros((128, 2), np.float32)
    m[0:32, 0] = 1.0
    m[64:96, 0] = 1.0
    m[32:64, 1] = 1.0
    m[96:128, 1] = 1.0

    # host-side hidden-state contribution: hb[b, :] = hid[b] @ W1[H:] + b1
    hb_all = hid[0] @ W1[H:] + b1                       # (B, H) f32
    ln_alpha = math.log(SELU_ALPHA)
    ln_sw = math.log(SW)

    in_maps = []
    for c in range(N_CORES):
        sl = slice(BL * c, BL * (c + 1))
        hb = hb_all[sl].reshape(BL, JC, 128).transpose(2, 1, 0)  # (128,JC,BL)
        # exp bias: hb + ln(alpha) (+ ln(SW) for odd j so e2 = SW*alpha*e^x)
        hbe = hb + ln_alpha + ln_sw * (np.arange(JC) % 2)[None, :, None]
        # relu bias: hb (ACT, even j) or SW*hb (DVE, odd j)
        hbr = hb * np.where(np.arange(JC) % 2 == 1, SW, 1.0)[None, :, None]
        in_maps.append({
            "enc": np.ascontiguousarray(enc[sl]).reshape(BL, TT, 128, H),
            "w1a8": w1a8,
            "w2lp": w2lp,
            "hbe": np.ascontiguousarray(hbe.astype(np.float32)),
            "hbr": np.ascontiguousarray(hbr.astype(np.float32)),
            "maskb": m.astype(bf16),
            "wcz": np.zeros((128, KC, 32), bf16),
        })
    return in_maps


def kernel(encoder_outputs, hidden_state, W1, b1, W2, b2):
    # b2 shifts every score equally; softmax is shift-invariant, so it is
    # deliberately unused.
    in_maps = make_in_maps(encoder_outputs, hidden_state, W1, b1, W2)
    nc = _get_nc()
    res = run_bass_kernel_spmd(nc, in_maps, core_ids=list(range(N_CORES)))
    out = np.empty((1, B, H), np.float32)
    for c in range(N_CORES):
        z = res.results[c]["zs"].sum(axis=1, keepdims=True)   # (BL, 1)
        p4 = res.results[c]["outp4"]                          # (BL, 4, 512)
        ctx = np.concatenate([p4[:, 0] + p4[:, 2],
                              p4[:, 1] + p4[:, 3]], axis=1)   # (BL, H)
        out[0, BL * c:BL * (c + 1)] = ctx / z
    return out


# revision 43
# speedup vs baseline: 1.1684x; 1.0245x over previous
"""Additive (Bahdanau-style) attention kernel for Trainium2, 8 NeuronCores.

reference computation (per batch b of 32, T=1024 timesteps, H=1024):
    mlp_hidden = selu([enc[b,t]; hid[b]] @ W1 + b1)     # (T, H)
    scores     = mlp_hidden @ W2 + b2                   # (T, 1)
    weights    = softmax(scores over t)
    out[b]     = sum_t weights[t] * enc[b,t]            # (H,)

Distribution: data-parallel over batch, 4 batches per core, no collectives.

Per-core algorithm (v3, fp8 DoubleRow, software-pipelined):
  - the hid @ W1[H:] + b1 term is per-batch constant; precomputed on the
    host and shipped as per-(j,b) bias columns (relu / exp variants).
  - scores path in fp8e4: E loaded once from HBM as bf16 (DMA cast),
    PE-transposed per 128x128 block, PSUM->SBUF copy casts to fp8.  W1a
    host-cast to fp8 with a 2^10 scale (entries ~1e-2 sit below fp8's
    normal range); the descale rides the ACT/DVE affine inputs.  The mlp
    matmul uses perf_mode=DoubleRow: one instruction contracts 256 rows.
  - selu = max(x,0) + min(alpha*e^x, alpha) (+const, dropped by softmax).
    exp always on ACT; the relu alternates between ACT (even j) and DVE
    tensor_scalar (odd j) to keep the j-loop from being ACT-bound.  For
    DVE j's the whole s2 is scaled by SW (host pre-scales that j's bias
    by SW and W2 column by 1/SW) so no extra scaling op is needed.
  - eT cast-copies alternate DVE / ACT per k for the same reason.
  - scores dot / context matmul have 1-wide outputs; packed 4-way into
    disjoint 32-column PE groups via tile_position (stationaries zero-
    padded to 32 cols so all PSUM partitions are written), then reduced
    across the 4 partial rows by a masked-ones PE matmul to partition 0.
  - software pipeline per batch b: loads(b+1) up front; transpose group
    tt of batch b+1 emitted after main-matmul group j=tt; epilogue(b-1)
    pieces emitted after j=4/5/6 so its matmuls land mid-stream (PE warm,
    no HAM re-throttle); score tail at the end of the j-loop.
  - softmax: exp off the reduced score rows (scores O(1), no max shift);
    1/Z folded into the output copy's scale.
"""

import math

import ml_dtypes
import numpy as np

import concourse.tile as tile
from concourse.masks import make_identity
from concourse import bacc, mybir
from concourse.bass_utils import run_bass_kernel_spmd

F32 = mybir.dt.float32
BF16 = mybir.dt.bfloat16
FP8 = mybir.dt.float8e4
ALU = mybir.AluOpType
ACTF = mybir.ActivationFunctionType
DR = mybir.MatmulPerfMode.DoubleRow

N_CORES = 8
B = 32
T = 1024
H = 1024
BL = B // N_CORES          # batches per core = 4
KC = H // 128              # contraction chunks = 8
JC = H // 128              # hidden-unit chunks = 8
TT = T // 128              # 128-row t-subtiles per batch = 8

SELU_LAMBDA = 1.0507009873554805
SELU_ALPHA = 1.6732632423543772
SW = 1024.0                # fp8 scale for W1a (and odd-j s2 scale)


def build_kernel():
    nc = bacc.Bacc("TRN2", target_bir_lowering=False, debug=False,
                   num_devices=N_CORES)

    enc = nc.dram_tensor("enc", [BL, TT, 128, H], F32, kind="ExternalInput").ap()
    w1a8 = nc.dram_tensor("w1a8", [KC, 128, H], FP8, kind="ExternalInput").ap()
    w2lp = nc.dram_tensor("w2lp", [128, JC, 32], BF16, kind="ExternalInput").ap()
    hbe = nc.dram_tensor("hbe", [128, JC, BL], F32, kind="ExternalInput").ap()
    hbr = nc.dram_tensor("hbr", [128, JC, BL], F32, kind="ExternalInput").ap()
    maskb = nc.dram_tensor("maskb", [128, 2], BF16, kind="ExternalInput").ap()
    wcz = nc.dram_tensor("wcz", [128, KC, 32], BF16, kind="ExternalInput").ap()
    outp4 = nc.dram_tensor("outp4", [BL, 4, 512], F32,
                           kind="ExternalOutput").ap()
    zs = nc.dram_tensor("zs", [BL, 2], F32, kind="ExternalOutput").ap()

    with tile.TileContext(nc) as tc:
        with (
            tc.tile_pool(name="consts", bufs=1) as consts,
            tc.tile_pool(name="encp", bufs=3) as encp,
            tc.tile_pool(name="etp", bufs=2) as etp,
            tc.tile_pool(name="selu", bufs=4) as selup,
            tc.tile_pool(name="score", bufs=2) as scorep,
            tc.tile_pool(name="outp", bufs=2) as outp,
            tc.tile_pool(name="psum", bufs=2, space="PSUM") as psum,
        ):
            # identity + PE warmup first: the warmup keeps the TensorE
            # activity monitor busy (so the clock ungates) while the first
            # batch's DMAs stream in.
            identity = consts.tile([128, 128], BF16)
            make_identity(nc, identity)
            one1 = consts.tile([1, 1], F32)
            nc.vector.memset(one1, 1.0)
            junk = consts.tile([128, 128], BF16)
            nc.vector.memset(junk, 0.0)
            warm_ps = psum.tile([128, 128], BF16, tag="sc", bufs=1)
            for _ in range(72):
                nc.tensor.transpose(warm_ps, junk, junk)

            # --- replicated weights / biases (sync queue, parallel with
            # the gpsimd encoder stream) --------------------------------
            w1a_sb = consts.tile([128, KC, H], FP8)
            nc.sync.dma_start(out=w1a_sb, in_=w1a8.rearrange("k p h -> p k h"))
            hbe_sb = consts.tile([128, JC, BL], F32)
            nc.sync.dma_start(out=hbe_sb, in_=hbe)
            hbr_sb = consts.tile([128, JC, BL], F32)
            nc.sync.dma_start(out=hbr_sb, in_=hbr)
            w2lp_sb = consts.tile([128, JC, 32], BF16)
            nc.sync.dma_start(out=w2lp_sb, in_=w2lp)
            maskb_sb = consts.tile([128, 2], BF16)
            nc.sync.dma_start(out=maskb_sb, in_=maskb)
            # zero-padded context-weight stationary: zeros come from the
            # host; each epilogue overwrites only column 0 of each block.
            wcol_pad = consts.tile([128, KC, 32], BF16)
            nc.sync.dma_start(out=wcol_pad, in_=wcz)

            def emit_loads(b):
                # gpsimd software-DGE DMAs cast f32 -> bf16 on the fly.
                e_ts = []
                for tt in range(TT):
                    e_t = encp.tile([128, H], BF16, tag="e", bufs=3 * TT,
                                    name=f"e_{b}_{tt}")
                    nc.gpsimd.dma_start(out=e_t, in_=enc[b, tt])
                    e_ts.append(e_t)
                return e_ts

            def alloc_eT(b):
                return etp.tile([128, KC, T], FP8, tag="eT", bufs=2,
                                name=f"eT_{b}")

            def emit_transpose_tt(b, e_ts, eT, tt):
                # all KC h-chunks of one t-subtile -> eT[:, :, tt*128:...]
                # (starts as soon as e_ts[tt]'s DMA lands).  The cast copy
                # alternates DVE/ACT to balance engine load.
                tp = psum.tile([128, KC, 128], BF16, tag="trans", bufs=2)
                for k in range(KC):
                    nc.tensor.transpose(
                        tp[:, k, :],
                        e_ts[tt][:, k * 128:(k + 1) * 128],
                        identity,
                    )
                dst = eT[:, :, tt * 128:(tt + 1) * 128]
                src = tp
                if tt % 2 == 0:
                    nc.vector.tensor_copy(out=dst, in_=src)
                else:
                    nc.scalar.activation(out=dst, in_=src, func=ACTF.Copy,
                                         scale=1.0)

            def emit_scores_tail(b, sc_ps):
                # stage partials in SBUF (bf16), PE-reduce per tg to
                # partition 0, exp with row-sum accumulation.
                scs = scorep.tile([128, 512], BF16, tag="scs")
                nc.scalar.activation(out=scs, in_=sc_ps, func=ACTF.Copy,
                                     scale=1.0)
                expw = scorep.tile([1, T], F32, tag="expw")
                rs2 = scorep.tile([1, 2], F32, tag="rsum2")
                for tg in range(2):
                    scr = psum.tile([1, 512], F32, tag="ctx", bufs=1)
                    nc.tensor.matmul(
                        scr,
                        lhsT=maskb_sb[:, tg:tg + 1],
                        rhs=scs,
                        start=True, stop=True,
                    )
                    nc.scalar.activation(
                        out=expw[:, tg * 512:(tg + 1) * 512], in_=scr,
                        func=ACTF.Exp, scale=1.0, accum_out=rs2[:, tg:tg + 1])
                return expw, rs2

            # ---- epilogue pieces (for batch whose phase1 has finished) --
            def epi_weights(state):
                # weights -> padded column stationary (PE transpose).
                e_ts, expw = state[:2]
                w_ps = psum.tile([128, KC, 1], F32, tag="ctx", bufs=1)
                for c in range(KC):
                    nc.tensor.transpose(
                        w_ps[:, c, :],
                        expw[0:1, c * 128:(c + 1) * 128],
                        one1,
                    )
                nc.vector.tensor_copy(out=wcol_pad[:, :, 0:1], in_=w_ps)

            def epi_context(state):
                # context[h] partials, col-group packed 4-way.
                e_ts = state[0]
                cp = psum.tile([128, 512], F32, tag="ctx", bufs=1)
                for half in range(2):
                    for tch in range(KC):
                        pos = 64 * (tch % 2) + 32 * half
                        nc.tensor.matmul(
                            cp[pos:pos + 32, :],
                            lhsT=wcol_pad[:, tch, :],
                            rhs=e_ts[tch][:, half * 512:(half + 1) * 512],
                            start=(tch < 2),
                            stop=(tch >= KC - 2),
                            tile_position=(0, pos),
                        )
                return cp

            def epi_out(b, cp, state):
                # unnormalized context partials out; the host sums the
                # col-group partial rows and divides by the softmax Z
                # (removes the masked-reduce matmuls + copies from the
                # serial tail).
                rs2 = state[2]
                cps = outp.tile([128, 512], F32, tag="cps")
                nc.scalar.activation(out=cps, in_=cp, func=ACTF.Copy,
                                     scale=1.0)
                nc.sync.dma_start(out=zs[b:b + 1, :], in_=rs2)
                nc.sync.dma_start(out=outp4[b], in_=cps[0:97:32, :])

            def phase1(b, e_ts, eT, next_ctx, prev_ctx, final=False):
                """Main pass for batch b.

                next_ctx: (e_ts, eT) of batch b+1 whose transposes are
                interleaved into this j-loop (or None).
                prev_ctx: (b-1, state) whose epilogue is interleaved
                (or None).
                """
                sc_ps = psum.tile([128, 512], F32, tag="sc", bufs=1)
                s2_prev = None
                epi = {}

                def emit_score(j, s2):
                    for tg in range(2):
                        pos = 64 * (j % 2) + 32 * tg
                        nc.tensor.matmul(
                            sc_ps[pos:pos + 32, :],
                            lhsT=w2lp_sb[:, j, :],
                            rhs=s2[:, tg * 512:(tg + 1) * 512],
                            start=(j < 2),
                            stop=(j >= JC - 2),
                            tile_position=(0, pos),
                        )

                for j in range(JC):
                    mp = psum.tile([128, T], F32, tag="mlp", bufs=2)
                    for tg in range(2):
                        for kk in range(KC // 2):
                            nc.tensor.matmul(
                                mp[:, tg * 512:(tg + 1) * 512],
                                lhsT=w1a_sb[:, 2 * kk:2 * kk + 2,
                                            j * 128:(j + 1) * 128],
                                rhs=eT[:, 2 * kk:2 * kk + 2,
                                       tg * 512:(tg + 1) * 512],
                                start=(kk == 0),
                                stop=(kk == KC // 2 - 1),
                                perf_mode=DR,
                            )
                    if next_ctx is not None:
                        # front-loaded so the last cast-copy lands before
                        # the next batch's first DoubleRow matmul.
                        for tt in ([j] if j < 5 else [5, 6] if j == 5
                                   else [7] if j == 6 else []):
                            emit_transpose_tt(b + 1, next_ctx[0],
                                              next_ctx[1], tt)
                    if prev_ctx is not None:
                        pb, pstate = prev_ctx
                        if j == 4:
                            epi_weights(pstate)
                        elif j == 5:
                            epi["cp"] = epi_context(pstate)
                        elif j == 6:
                            epi_out(pb, epi["cp"], pstate)
                    if s2_prev is not None:
                        emit_score(j - 1, s2_prev)
                    e2 = selup.tile([128, T], BF16, tag="e2")
                    nc.scalar.activation(out=e2, in_=mp, func=ACTF.Exp,
                                         bias=hbe_sb[:, j, b:b + 1],
                                         scale=1.0 / SW)
                    r2 = selup.tile([128, T], BF16, tag="r2")
                    if j % 2 == 0:
                        # ACT path: r2 = relu(mp/SW + hb)
                        nc.scalar.activation(out=r2, in_=mp, func=ACTF.Relu,
                                             bias=hbr_sb[:, j, b:b + 1],
                                             scale=1.0 / SW)
                        alpha_cap = SELU_ALPHA
                    else:
                        # DVE path, SW-scaled: r2 = max(mp + SW*hb, 0);
                        # this j's whole s2 is scaled by SW (host divides
                        # the W2 column by SW and offsets the exp bias).
                        nc.vector.tensor_scalar(
                            out=r2, in0=mp, scalar1=hbr_sb[:, j, b:b + 1],
                            scalar2=0.0, op0=ALU.add, op1=ALU.max,
                        )
                        alpha_cap = SELU_ALPHA * SW
                    # s2 = min(e2, alpha) + r2, single fused DVE op
                    s2 = selup.tile([128, T], BF16, tag="s2", bufs=4)
                    nc.vector.scalar_tensor_tensor(
                        out=s2, in0=e2, scalar=alpha_cap, in1=r2,
                        op0=ALU.min, op1=ALU.add,
                    )
                    s2_prev = s2
                emit_score(JC - 1, s2_prev)
                if final:
                    # stage the partials; the pipelined tail does the rest
                    nonlocal last_scs
                    last_scs = scorep.tile([128, 512], BF16, tag="scs")
                    nc.scalar.activation(out=last_scs, in_=sc_ps,
                                         func=ACTF.Copy, scale=1.0)
                    expw = scorep.tile([1, T], F32, tag="expw")
                    rs2 = scorep.tile([1, 2], F32, tag="rsum2")
                    return (e_ts, expw, rs2)
                expw, rs2 = emit_scores_tail(b, sc_ps)
                return (e_ts, expw, rs2)

            def selu_chain(b, j, mp, width):
                """e2/r2/s2 for one (j, tg-or-full) mp tile; returns s2."""
                e2 = selup.tile([128, width], BF16, tag="e2")
                nc.scalar.activation(out=e2, in_=mp, func=ACTF.Exp,
                                     bias=hbe_sb[:, j, b:b + 1],
                                     scale=1.0 / SW)
                r2 = selup.tile([128, width], BF16, tag="r2")
                if j % 2 == 0:
                    nc.scalar.activation(out=r2, in_=mp, func=ACTF.Relu,
                                         bias=hbr_sb[:, j, b:b + 1],
                                         scale=1.0 / SW)
                    alpha_cap = SELU_ALPHA
                else:
                    nc.vector.tensor_scalar(
                        out=r2, in0=mp, scalar1=hbr_sb[:, j, b:b + 1],
                        scalar2=0.0, op0=ALU.add, op1=ALU.max,
                    )
                    alpha_cap = SELU_ALPHA * SW
                s2 = selup.tile([128, width], BF16, tag="s2", bufs=4)
                nc.vector.scalar_tensor_tensor(
                    out=s2, in0=e2, scalar=alpha_cap, in1=r2,
                    op0=ALU.min, op1=ALU.add,
                )
                return s2

            def phase1_split(b, e_ts, eT, next_ctx, prev_ctx, own_tail):
                """Per-t-group pass for the first / last batch.

                b==0: tg0's j-loop only needs transpose groups tt0-3, so it
                starts ~4 DMAs earlier; own tt4-7 and batch 1's transposes
                are interleaved.  b==BL-1: tg0's softmax / weight transpose
                / first context half overlap tg1's j-loop so the serial
                tail shrinks.  Scores here accumulate sequentially into a
                [1,512] row per tg (no staging/reduce hop).
                """
                expw = scorep.tile([1, T], F32, tag="expw")
                rs2 = scorep.tile([1, 2], F32, tag="rsum2")
                epi = {}

                for tg in range(2):
                    sc_row = psum.tile([1, 512], F32, tag="sc", bufs=1)
                    tgs = slice(tg * 512, (tg + 1) * 512)
                    s2q = []

                    def emit_score_seq(j, s2):
                        nc.tensor.matmul(
                            sc_row,
                            lhsT=w2lp_sb[:, j, 0:1],
                            rhs=s2,
                            start=(j == 0),
                            stop=(j == JC - 1),
                        )

                    for j in range(JC):
                        mp = psum.tile([128, 512], F32, tag="mlp", bufs=2)
                        for kk in range(KC // 2):
                            nc.tensor.matmul(
                                mp,
                                lhsT=w1a_sb[:, 2 * kk:2 * kk + 2,
                                            j * 128:(j + 1) * 128],
                                rhs=eT[:, 2 * kk:2 * kk + 2, tgs],
                                start=(kk == 0),
                                stop=(kk == KC // 2 - 1),
                                perf_mode=DR,
                            )
                        if own_tail and tg == 0 and prev_ctx is not None:
                            pb, pstate = prev_ctx
                            if j == 4:
                                epi_weights(pstate)
                            elif j == 5:
                                epi["pcp"] = epi_context(pstate)
                            elif j == 6:
                                epi_out(pb, epi["pcp"], pstate)
                        if own_tail and tg == 1:
                            if j == 0:
                                # softmax of tg0 while tg1 streams
                                nc.scalar.activation(
                                    out=expw[:, 0:512], in_=epi["sc0"],
                                    func=ACTF.Exp, scale=1.0,
                                    accum_out=rs2[:, 0:1])
                            elif j == 2:
                                wps_a = psum.tile([128, 4, 1], F32,
                                                  tag="ctx", bufs=1)
                                for c in range(4):
                                    nc.tensor.transpose(
                                        wps_a[:, c, :],
                                        expw[0:1, c * 128:(c + 1) * 128],
                                        one1,
                                    )
                                nc.vector.tensor_copy(
                                    out=wcol_pad[:, 0:4, 0:1], in_=wps_a)
                            elif j == 4:
                                cp = psum.tile([128, 512], F32, tag="ctx",
                                               bufs=1)
                                epi["cp"] = cp
                                for half in range(2):
                                    for tch in range(4):
                                        pos = 64 * (tch % 2) + 32 * half
                                        nc.tensor.matmul(
                                            cp[pos:pos + 32, :],
                                            lhsT=wcol_pad[:, tch, :],
                                            rhs=e_ts[tch][:, half * 512:
                                                          (half + 1) * 512],
                                            start=(tch < 2),
                                            stop=False,
                                            tile_position=(0, pos),
                                        )
                        # deep score deferral (j-3): at half-width pacing
                        # the selu chain needs ~3 PE j-steps of slack before
                        # the score matmul consumes its s2
                        if len(s2q) >= 3:
                            emit_score_seq(j - 3, s2q[-3])
                        s2q.append(selu_chain(b, j, mp, 512))
                    for jr in (JC - 3, JC - 2, JC - 1):
                        emit_score_seq(jr, s2q[jr])
                    if tg == 0:
                        epi["sc0"] = sc_row
                        if not own_tail:
                            nc.scalar.activation(
                                out=expw[:, 0:512], in_=sc_row,
                                func=ACTF.Exp, scale=1.0,
                                accum_out=rs2[:, 0:1])
                    else:
                        nc.scalar.activation(
                            out=expw[:, 512:1024], in_=sc_row,
                            func=ACTF.Exp, scale=1.0, accum_out=rs2[:, 1:2])

                if not own_tail:
                    return (e_ts, expw, rs2)

                # remaining tail: weight transposes c4-7, context tch4-7,
                # reduce, out
                wps_b = psum.tile([128, 4, 1], F32, tag="sc", bufs=1)
                for c in range(4):
                    nc.tensor.transpose(
                        wps_b[:, c, :],
                        expw[0:1, (c + 4) * 128:(c + 5) * 128],
                        one1,
                    )
                nc.vector.tensor_copy(out=wcol_pad[:, 4:8, 0:1], in_=wps_b)
                cp = epi["cp"]
                for half in range(2):
                    for tch in range(4, KC):
                        pos = 64 * (tch % 2) + 32 * half
                        nc.tensor.matmul(
                            cp[pos:pos + 32, :],
                            lhsT=wcol_pad[:, tch, :],
                            rhs=e_ts[tch][:, half * 512:(half + 1) * 512],
                            start=False,
                            stop=(tch >= KC - 2),
                            tile_position=(0, pos),
                        )
                epi_out(b, cp, (e_ts, expw, rs2))
                return None

            # ---------------- top-level software pipeline ----------------
            e_ts = emit_loads(0)
            eT = alloc_eT(0)
            for tt in range(TT):
                emit_transpose_tt(0, e_ts, eT, tt)

            prev_state = None
            for b in range(0, BL - 1):
                e_ts_n = emit_loads(b + 1)
                next_ctx = (e_ts_n, alloc_eT(b + 1))
                prev_ctx = (b - 1, prev_state) if prev_state is not None \
                    else None
                state = phase1(b, e_ts, eT, next_ctx, prev_ctx)
                prev_state = state
                e_ts, eT = next_ctx

            last_scs = None
            state = phase1(BL - 1, e_ts, eT, None, (BL - 2, prev_state),
                           final=True)
            # pipelined final tail: the second reduce rides the freed sc
            # bank so both reduces issue back-to-back, and each half's
            # weight transposes run between the two exps.
            e_ts_l, expw_l, rs2_l = state
            scr0 = psum.tile([1, 512], F32, tag="ctx", bufs=1)
            nc.tensor.matmul(scr0, lhsT=maskb_sb[:, 0:1], rhs=last_scs,
                             start=True, stop=True)
            scr1 = psum.tile([1, 512], F32, tag="sc", bufs=1)
            nc.tensor.matmul(scr1, lhsT=maskb_sb[:, 1:2], rhs=last_scs,
                             start=True, stop=True)
            nc.scalar.activation(out=expw_l[:, 0:512], in_=scr0,
                                 func=ACTF.Exp, scale=1.0,
                                 accum_out=rs2_l[:, 0:1])
            wps_a = psum.tile([128, 4, 1], F32, tag="ctx", bufs=1)
            for c in range(4):
                nc.tensor.transpose(wps_a[:, c, :],
                                    expw_l[0:1, c * 128:(c + 1) * 128], one1)
            nc.vector.tensor_copy(out=wcol_pad[:, 0:4, 0:1], in_=wps_a)
            nc.scalar.activation(out=expw_l[:, 512:1024], in_=scr1,
                                 func=ACTF.Exp, scale=1.0,
                                 accum_out=rs2_l[:, 1:2])
            wps_b = psum.tile([128, 4, 1], F32, tag="sc", bufs=1)
            for c in range(4):
                nc.tensor.transpose(
                    wps_b[:, c, :],
                    expw_l[0:1, (c + 4) * 128:(c + 5) * 128], one1)
            nc.vector.tensor_copy(out=wcol_pad[:, 4:8, 0:1], in_=wps_b)
            cp = epi_context(state)
            epi_out(BL - 1, cp, state)

    nc.compile()
    return nc


_NC_CACHE = None


def _get_nc():
    global _NC_CACHE
    if _NC_CACHE is None:
        _NC_CACHE = build_kernel()
    return _NC_CACHE


def make_in_maps(encoder_outputs, hidden_state, W1, b1, W2):
    enc = np.ascontiguousarray(np.asarray(encoder_outputs, np.float32))
    hid = np.ascontiguousarray(np.asarray(hidden_state, np.float32))
    W1 = np.asarray(W1, np.float32)
    b1 = np.asarray(b1, np.float32)
    W2 = np.asarray(W2, np.float32)

    bf16 = ml_dtypes.bfloat16
    f8 = ml_dtypes.float8_e4m3
    # cast to the HW e4m3 format, but ship the bytes under the e4m3fn
    # container dtype: the PJRT path rejects the IEEE f8E4M3 HLO type
    # while accepting f8E4M3FN, and bass's input check is fuzzy across
    # the two.
    w1a8 = np.ascontiguousarray(
        (W1[:H] * SW).reshape(KC, 128, H)).astype(f8).view(
            ml_dtypes.float8_e4m3fn)

    # per-j s2 scale: odd j's selu output is scaled by SW (DVE relu path)
    jscale = np.where(np.arange(JC) % 2 == 1, SW, 1.0).astype(np.float32)
    w2l = (W2[:, 0] * SELU_LAMBDA).reshape(JC, 128) / jscale[:, None]
    w2lp = np.zeros((128, JC, 32), bf16)
    w2lp[:, :, 0] = w2l.T.astype(bf16)

    # reduction masks: tg0/half0 partials live at partitions [0,32) and
    # [64,96) (real rows 0 and 64, zeros elsewhere), tg1/half1 at the
    # complement.
    m = np.zeros((128, 2), np.float32)
    m[0:32, 0] = 1.0
    m[64:96, 0] = 1.0
    m[32:64, 1] = 1.0
    m[96:128, 1] = 1.0

    # host-side hidden-state contribution: hb[b, :] = hid[b] @ W1[H:] + b1
    hb_all = hid[0] @ W1[H:] + b1                       # (B, H) f32
    ln_alpha = math.log(SELU_ALPHA)
    ln_sw = math.log(SW)

    in_maps = []
    for c in range(N_CORES):
        sl = slice(BL * c, BL * (c + 1))
        hb = hb_all[sl].reshape(BL, JC, 128).transpose(2, 1, 0)  # (128,JC,BL)
        # exp bias: hb + ln(alpha) (+ ln(SW) for odd j so e2 = SW*alpha*e^x)
        hbe = hb + ln_alpha + ln_sw * (np.arange(JC) % 2)[None, :, None]
        # relu bias: hb (ACT, even j) or SW*hb (DVE, odd j)
        hbr = hb * np.where(np.arange(JC) % 2 == 1, SW, 1.0)[None, :, None]
        in_maps.append({
            "enc": np.ascontiguousarray(enc[sl]).reshape(BL, TT, 128, H),
            "w1a8": w1a8,
            "w2lp": w2lp,
            "hbe": np.ascontiguousarray(hbe.astype(np.float32)),
            "hbr": np.ascontiguousarray(hbr.astype(np.float32)),
            "maskb": m.astype(bf16),
            "wcz": np.zeros((128, KC, 32), bf16),
        })
    return in_maps


def kernel(encoder_outputs, hidden_state, W1, b1, W2, b2):
    # b2 shifts every score equally; softmax is shift-invariant, so it is
    # deliberately unused.
    in_maps = make_in_maps(encoder_outputs, hidden_state, W1, b1, W2)
    nc = _get_nc()
    res = run_bass_kernel_spmd(nc, in_maps, core_ids=list(range(N_CORES)))
    out = np.empty((1, B, H), np.float32)
    for c in range(N_CORES):
        z = res.results[c]["zs"].sum(axis=1, keepdims=True)   # (BL, 1)
        p4 = res.results[c]["outp4"]                          # (BL, 4, 512)
        ctx = np.concatenate([p4[:, 0] + p4[:, 2],
                              p4[:, 1] + p4[:, 3]], axis=1)   # (BL, H)
        out[0, BL * c:BL * (c + 1)] = ctx / z
    return out


# revision 44
# speedup vs baseline: 1.1734x; 1.0042x over previous
"""Additive (Bahdanau-style) attention kernel for Trainium2, 8 NeuronCores.

reference computation (per batch b of 32, T=1024 timesteps, H=1024):
    mlp_hidden = selu([enc[b,t]; hid[b]] @ W1 + b1)     # (T, H)
    scores     = mlp_hidden @ W2 + b2                   # (T, 1)
    weights    = softmax(scores over t)
    out[b]     = sum_t weights[t] * enc[b,t]            # (H,)

Distribution: data-parallel over batch, 4 batches per core, no collectives.

Per-core algorithm (v3, fp8 DoubleRow, software-pipelined):
  - the hid @ W1[H:] + b1 term is per-batch constant; precomputed on the
    host and shipped as per-(j,b) bias columns (relu / exp variants).
  - scores path in fp8e4: E loaded once from HBM as bf16 (DMA cast),
    PE-transposed per 128x128 block, PSUM->SBUF copy casts to fp8.  W1a
    host-cast to fp8 with a 2^10 scale (entries ~1e-2 sit below fp8's
    normal range); the descale rides the ACT/DVE affine inputs.  The mlp
    matmul uses perf_mode=DoubleRow: one instruction contracts 256 rows.
  - selu = max(x,0) + min(alpha*e^x, alpha) (+const, dropped by softmax).
    exp always on ACT; the relu alternates between ACT (even j) and DVE
    tensor_scalar (odd j) to keep the j-loop from being ACT-bound.  For
    DVE j's the whole s2 is scaled by SW (host pre-scales that j's bias
    by SW and W2 column by 1/SW) so no extra scaling op is needed.
  - eT cast-copies alternate DVE / ACT per k for the same reason.
  - scores dot / context matmul have 1-wide outputs; packed 4-way into
    disjoint 32-column PE groups via tile_position (stationaries zero-
    padded to 32 cols so all PSUM partitions are written), then reduced
    across the 4 partial rows by a masked-ones PE matmul to partition 0.
  - software pipeline per batch b: loads(b+1) up front; transpose group
    tt of batch b+1 emitted after main-matmul group j=tt; epilogue(b-1)
    pieces emitted after j=4/5/6 so its matmuls land mid-stream (PE warm,
    no HAM re-throttle); score tail at the end of the j-loop.
  - softmax: exp off the reduced score rows (scores O(1), no max shift);
    1/Z folded into the output copy's scale.
"""

import math

import ml_dtypes
import numpy as np

import concourse.tile as tile
from concourse.masks import make_identity
from concourse import bacc, mybir
from concourse.bass_utils import run_bass_kernel_spmd

F32 = mybir.dt.float32
BF16 = mybir.dt.bfloat16
FP8 = mybir.dt.float8e4
ALU = mybir.AluOpType
ACTF = mybir.ActivationFunctionType
DR = mybir.MatmulPerfMode.DoubleRow

N_CORES = 8
B = 32
T = 1024
H = 1024
BL = B // N_CORES          # batches per core = 4
KC = H // 128              # contraction chunks = 8
JC = H // 128              # hidden-unit chunks = 8
TT = T // 128              # 128-row t-subtiles per batch = 8

SELU_LAMBDA = 1.0507009873554805
SELU_ALPHA = 1.6732632423543772
SW = 1024.0                # fp8 scale for W1a (and odd-j s2 scale)


def build_kernel():
    nc = bacc.Bacc("TRN2", target_bir_lowering=False, debug=False,
                   num_devices=N_CORES)

    enc = nc.dram_tensor("enc", [BL, TT, 128, H], F32, kind="ExternalInput").ap()
    w1a8 = nc.dram_tensor("w1a8", [KC, 128, H], FP8, kind="ExternalInput").ap()
    w2lp = nc.dram_tensor("w2lp", [128, JC, 32], BF16, kind="ExternalInput").ap()
    hbe = nc.dram_tensor("hbe", [128, JC, BL], F32, kind="ExternalInput").ap()
    hbr = nc.dram_tensor("hbr", [128, JC, BL], F32, kind="ExternalInput").ap()
    maskb = nc.dram_tensor("maskb", [128, 2], BF16, kind="ExternalInput").ap()
    wcz = nc.dram_tensor("wcz", [128, KC, 32], BF16, kind="ExternalInput").ap()
    outp4 = nc.dram_tensor("outp4", [BL, 4, 512], F32,
                           kind="ExternalOutput").ap()
    zs = nc.dram_tensor("zs", [BL, 2], F32, kind="ExternalOutput").ap()

    with tile.TileContext(nc) as tc:
        with (
            tc.tile_pool(name="consts", bufs=1) as consts,
            tc.tile_pool(name="encp", bufs=3) as encp,
            tc.tile_pool(name="etp", bufs=2) as etp,
            tc.tile_pool(name="selu", bufs=4) as selup,
            tc.tile_pool(name="score", bufs=2) as scorep,
            tc.tile_pool(name="outp", bufs=2) as outp,
            tc.tile_pool(name="psum", bufs=2, space="PSUM") as psum,
        ):
            # identity + PE warmup first: the warmup keeps the TensorE
            # activity monitor busy (so the clock ungates) while the first
            # batch's DMAs stream in.
            identity = consts.tile([128, 128], BF16)
            make_identity(nc, identity)
            one1 = consts.tile([1, 1], F32)
            nc.vector.memset(one1, 1.0)
            junk = consts.tile([128, 128], BF16)
            nc.vector.memset(junk, 0.0)
            warm_ps = psum.tile([128, 128], BF16, tag="sc", bufs=1)
            for _ in range(72):
                nc.tensor.transpose(warm_ps, junk, junk)

            # --- replicated weights / biases (sync queue, parallel with
            # the gpsimd encoder stream) --------------------------------
            w1a_sb = consts.tile([128, KC, H], FP8)
            nc.sync.dma_start(out=w1a_sb, in_=w1a8.rearrange("k p h -> p k h"))
            hbe_sb = consts.tile([128, JC, BL], F32)
            nc.sync.dma_start(out=hbe_sb, in_=hbe)
            hbr_sb = consts.tile([128, JC, BL], F32)
            nc.sync.dma_start(out=hbr_sb, in_=hbr)
            w2lp_sb = consts.tile([128, JC, 32], BF16)
            nc.sync.dma_start(out=w2lp_sb, in_=w2lp)
            maskb_sb = consts.tile([128, 2], BF16)
            nc.sync.dma_start(out=maskb_sb, in_=maskb)
            # zero-padded context-weight stationary: zeros come from the
            # host; each epilogue overwrites only column 0 of each block.
            wcol_pad = consts.tile([128, KC, 32], BF16)
            nc.sync.dma_start(out=wcol_pad, in_=wcz)

            def emit_loads(b):
                # gpsimd software-DGE DMAs cast f32 -> bf16 on the fly.
                e_ts = []
                for tt in range(TT):
                    e_t = encp.tile([128, H], BF16, tag="e", bufs=3 * TT,
                                    name=f"e_{b}_{tt}")
                    nc.gpsimd.dma_start(out=e_t, in_=enc[b, tt])
                    e_ts.append(e_t)
                return e_ts

            def alloc_eT(b):
                return etp.tile([128, KC, T], FP8, tag="eT", bufs=2,
                                name=f"eT_{b}")

            def emit_transpose_tt(b, e_ts, eT, tt):
                # all KC h-chunks of one t-subtile -> eT[:, :, tt*128:...]
                # (starts as soon as e_ts[tt]'s DMA lands).  The cast copy
                # alternates DVE/ACT to balance engine load.
                tp = psum.tile([128, KC, 128], BF16, tag="trans", bufs=2)
                for k in range(KC):
                    nc.tensor.transpose(
                        tp[:, k, :],
                        e_ts[tt][:, k * 128:(k + 1) * 128],
                        identity,
                    )
                dst = eT[:, :, tt * 128:(tt + 1) * 128]
                src = tp
                if tt == 7:
                    # tt7's copy gates the next batch's first matmul at the
                    # phase seam: split it by k-chunk across both engines so
                    # the early-needed chunks (kk=0,1) land first on DVE
                    # while ACT (busy with the phase tail) does the rest.
                    nc.vector.tensor_copy(out=eT[:, 0:4, tt * 128:(tt + 1) * 128],
                                          in_=tp[:, 0:4, :])
                    nc.scalar.activation(out=eT[:, 4:8, tt * 128:(tt + 1) * 128],
                                         in_=tp[:, 4:8, :], func=ACTF.Copy,
                                         scale=1.0)
                elif tt % 2 == 0:
                    nc.vector.tensor_copy(out=dst, in_=src)
                else:
                    nc.scalar.activation(out=dst, in_=src, func=ACTF.Copy,
                                         scale=1.0)

            def emit_scores_tail(b, sc_ps):
                # stage partials in SBUF (bf16), PE-reduce per tg to
                # partition 0, exp with row-sum accumulation.
                scs = scorep.tile([128, 512], BF16, tag="scs")
                nc.scalar.activation(out=scs, in_=sc_ps, func=ACTF.Copy,
                                     scale=1.0)
                expw = scorep.tile([1, T], F32, tag="expw")
                rs2 = scorep.tile([1, 2], F32, tag="rsum2")
                for tg in range(2):
                    scr = psum.tile([1, 512], F32, tag="ctx", bufs=1)
                    nc.tensor.matmul(
                        scr,
                        lhsT=maskb_sb[:, tg:tg + 1],
                        rhs=scs,
                        start=True, stop=True,
                    )
                    nc.scalar.activation(
                        out=expw[:, tg * 512:(tg + 1) * 512], in_=scr,
                        func=ACTF.Exp, scale=1.0, accum_out=rs2[:, tg:tg + 1])
                return expw, rs2

            # ---- epilogue pieces (for batch whose phase1 has finished) --
            def epi_weights(state):
                # weights -> padded column stationary (PE transpose).
                e_ts, expw = state[:2]
                w_ps = psum.tile([128, KC, 1], F32, tag="ctx", bufs=1)
                for c in range(KC):
                    nc.tensor.transpose(
                        w_ps[:, c, :],
                        expw[0:1, c * 128:(c + 1) * 128],
                        one1,
                    )
                nc.vector.tensor_copy(out=wcol_pad[:, :, 0:1], in_=w_ps)

            def epi_context(state):
                # context[h] partials, col-group packed 4-way.
                e_ts = state[0]
                cp = psum.tile([128, 512], F32, tag="ctx", bufs=1)
                for half in range(2):
                    for tch in range(KC):
                        pos = 64 * (tch % 2) + 32 * half
                        nc.tensor.matmul(
                            cp[pos:pos + 32, :],
                            lhsT=wcol_pad[:, tch, :],
                            rhs=e_ts[tch][:, half * 512:(half + 1) * 512],
                            start=(tch < 2),
                            stop=(tch >= KC - 2),
                            tile_position=(0, pos),
                        )
                return cp

            def epi_out(b, cp, state):
                # unnormalized context partials out; the host sums the
                # col-group partial rows and divides by the softmax Z
                # (removes the masked-reduce matmuls + copies from the
                # serial tail).
                rs2 = state[2]
                cps = outp.tile([128, 512], F32, tag="cps")
                nc.scalar.activation(out=cps, in_=cp, func=ACTF.Copy,
                                     scale=1.0)
                nc.sync.dma_start(out=zs[b:b + 1, :], in_=rs2)
                nc.sync.dma_start(out=outp4[b], in_=cps[0:97:32, :])

            def phase1(b, e_ts, eT, next_ctx, prev_ctx, final=False):
                """Main pass for batch b.

                next_ctx: (e_ts, eT) of batch b+1 whose transposes are
                interleaved into this j-loop (or None).
                prev_ctx: (b-1, state) whose epilogue is interleaved
                (or None).
                """
                sc_ps = psum.tile([128, 512], F32, tag="sc", bufs=1)
                s2_prev = None
                epi = {}

                def emit_score(j, s2):
                    for tg in range(2):
                        pos = 64 * (j % 2) + 32 * tg
                        nc.tensor.matmul(
                            sc_ps[pos:pos + 32, :],
                            lhsT=w2lp_sb[:, j, :],
                            rhs=s2[:, tg * 512:(tg + 1) * 512],
                            start=(j < 2),
                            stop=(j >= JC - 2),
                            tile_position=(0, pos),
                        )

                for j in range(JC):
                    mp = psum.tile([128, T], F32, tag="mlp", bufs=2)
                    for tg in range(2):
                        for kk in range(KC // 2):
                            nc.tensor.matmul(
                                mp[:, tg * 512:(tg + 1) * 512],
                                lhsT=w1a_sb[:, 2 * kk:2 * kk + 2,
                                            j * 128:(j + 1) * 128],
                                rhs=eT[:, 2 * kk:2 * kk + 2,
                                       tg * 512:(tg + 1) * 512],
                                start=(kk == 0),
                                stop=(kk == KC // 2 - 1),
                                perf_mode=DR,
                            )
                    if next_ctx is not None:
                        # front-loaded so the last cast-copy lands before
                        # the next batch's first DoubleRow matmul.
                        for tt in ([j] if j < 5 else [5, 6] if j == 5
                                   else [7] if j == 6 else []):
                            emit_transpose_tt(b + 1, next_ctx[0],
                                              next_ctx[1], tt)
                    if prev_ctx is not None:
                        pb, pstate = prev_ctx
                        if j == 4:
                            epi_weights(pstate)
                        elif j == 5:
                            epi["cp"] = epi_context(pstate)
                        elif j == 6:
                            epi_out(pb, epi["cp"], pstate)
                    if s2_prev is not None:
                        emit_score(j - 1, s2_prev)
                    e2 = selup.tile([128, T], BF16, tag="e2")
                    nc.scalar.activation(out=e2, in_=mp, func=ACTF.Exp,
                                         bias=hbe_sb[:, j, b:b + 1],
                                         scale=1.0 / SW)
                    r2 = selup.tile([128, T], BF16, tag="r2")
                    if j % 2 == 0:
                        # ACT path: r2 = relu(mp/SW + hb)
                        nc.scalar.activation(out=r2, in_=mp, func=ACTF.Relu,
                                             bias=hbr_sb[:, j, b:b + 1],
                                             scale=1.0 / SW)
                        alpha_cap = SELU_ALPHA
                    else:
                        # DVE path, SW-scaled: r2 = max(mp + SW*hb, 0);
                        # this j's whole s2 is scaled by SW (host divides
                        # the W2 column by SW and offsets the exp bias).
                        nc.vector.tensor_scalar(
                            out=r2, in0=mp, scalar1=hbr_sb[:, j, b:b + 1],
                            scalar2=0.0, op0=ALU.add, op1=ALU.max,
                        )
                        alpha_cap = SELU_ALPHA * SW
                    # s2 = min(e2, alpha) + r2, single fused DVE op
                    s2 = selup.tile([128, T], BF16, tag="s2", bufs=4)
                    nc.vector.scalar_tensor_tensor(
                        out=s2, in0=e2, scalar=alpha_cap, in1=r2,
                        op0=ALU.min, op1=ALU.add,
                    )
                    s2_prev = s2
                emit_score(JC - 1, s2_prev)
                if final:
                    # stage the partials; the pipelined tail does the rest
                    nonlocal last_scs
                    last_scs = scorep.tile([128, 512], BF16, tag="scs")
                    nc.scalar.activation(out=last_scs, in_=sc_ps,
                                         func=ACTF.Copy, scale=1.0)
                    expw = scorep.tile([1, T], F32, tag="expw")
                    rs2 = scorep.tile([1, 2], F32, tag="rsum2")
                    return (e_ts, expw, rs2)
                expw, rs2 = emit_scores_tail(b, sc_ps)
                return (e_ts, expw, rs2)

            def selu_chain(b, j, mp, width):
                """e2/r2/s2 for one (j, tg-or-full) mp tile; returns s2."""
                e2 = selup.tile([128, width], BF16, tag="e2")
                nc.scalar.activation(out=e2, in_=mp, func=ACTF.Exp,
                                     bias=hbe_sb[:, j, b:b + 1],
                                     scale=1.0 / SW)
                r2 = selup.tile([128, width], BF16, tag="r2")
                if j % 2 == 0:
                    nc.scalar.activation(out=r2, in_=mp, func=ACTF.Relu,
                                         bias=hbr_sb[:, j, b:b + 1],
                                         scale=1.0 / SW)
                    alpha_cap = SELU_ALPHA
                else:
                    nc.vector.tensor_scalar(
                        out=r2, in0=mp, scalar1=hbr_sb[:, j, b:b + 1],
                        scalar2=0.0, op0=ALU.add, op1=ALU.max,
                    )
                    alpha_cap = SELU_ALPHA * SW
                s2 = selup.tile([128, width], BF16, tag="s2", bufs=4)
                nc.vector.scalar_tensor_tensor(
                    out=s2, in0=e2, scalar=alpha_cap, in1=r2,
                    op0=ALU.min, op1=ALU.add,
                )
                return s2

            def phase1_split(b, e_ts, eT, next_ctx, prev_ctx, own_tail):
                """Per-t-group pass for the first / last batch.

                b==0: tg0's j-loop only needs transpose groups tt0-3, so it
                starts ~4 DMAs earlier; own tt4-7 and batch 1's transposes
                are interleaved.  b==BL-1: tg0's softmax / weight transpose
                / first context half overlap tg1's j-loop so the serial
                tail shrinks.  Scores here accumulate sequentially into a
                [1,512] row per tg (no staging/reduce hop).
                """
                expw = scorep.tile([1, T], F32, tag="expw")
                rs2 = scorep.tile([1, 2], F32, tag="rsum2")
                epi = {}

                for tg in range(2):
                    sc_row = psum.tile([1, 512], F32, tag="sc", bufs=1)
                    tgs = slice(tg * 512, (tg + 1) * 512)
                    s2q = []

                    def emit_score_seq(j, s2):
                        nc.tensor.matmul(
                            sc_row,
                            lhsT=w2lp_sb[:, j, 0:1],
                            rhs=s2,
                            start=(j == 0),
                            stop=(j == JC - 1),
                        )

                    for j in range(JC):
                        mp = psum.tile([128, 512], F32, tag="mlp", bufs=2)
                        for kk in range(KC // 2):
                            nc.tensor.matmul(
                                mp,
                                lhsT=w1a_sb[:, 2 * kk:2 * kk + 2,
                                            j * 128:(j + 1) * 128],
                                rhs=eT[:, 2 * kk:2 * kk + 2, tgs],
                                start=(kk == 0),
                                stop=(kk == KC // 2 - 1),
                                perf_mode=DR,
                            )
                        if own_tail and tg == 0 and prev_ctx is not None:
                            pb, pstate = prev_ctx
                            if j == 4:
                                epi_weights(pstate)
                            elif j == 5:
                                epi["pcp"] = epi_context(pstate)
                            elif j == 6:
                                epi_out(pb, epi["pcp"], pstate)
                        if own_tail and tg == 1:
                            if j == 0:
                                # softmax of tg0 while tg1 streams
                                nc.scalar.activation(
                                    out=expw[:, 0:512], in_=epi["sc0"],
                                    func=ACTF.Exp, scale=1.0,
                                    accum_out=rs2[:, 0:1])
                            elif j == 2:
                                wps_a = psum.tile([128, 4, 1], F32,
                                                  tag="ctx", bufs=1)
                                for c in range(4):
                                    nc.tensor.transpose(
                                        wps_a[:, c, :],
                                        expw[0:1, c * 128:(c + 1) * 128],
                                        one1,
                                    )
                                nc.vector.tensor_copy(
                                    out=wcol_pad[:, 0:4, 0:1], in_=wps_a)
                            elif j == 4:
                                cp = psum.tile([128, 512], F32, tag="ctx",
                                               bufs=1)
                                epi["cp"] = cp
                                for half in range(2):
                                    for tch in range(4):
                                        pos = 64 * (tch % 2) + 32 * half
                                        nc.tensor.matmul(
                                            cp[pos:pos + 32, :],
                                            lhsT=wcol_pad[:, tch, :],
                                            rhs=e_ts[tch][:, half * 512:
                                                          (half + 1) * 512],
                                            start=(tch < 2),
                                            stop=False,
                                            tile_position=(0, pos),
                                        )
                        # deep score deferral (j-3): at half-width pacing
                        # the selu chain needs ~3 PE j-steps of slack before
                        # the score matmul consumes its s2
                        if len(s2q) >= 3:
                            emit_score_seq(j - 3, s2q[-3])
                        s2q.append(selu_chain(b, j, mp, 512))
                    for jr in (JC - 3, JC - 2, JC - 1):
                        emit_score_seq(jr, s2q[jr])
                    if tg == 0:
                        epi["sc0"] = sc_row
                        if not own_tail:
                            nc.scalar.activation(
                                out=expw[:, 0:512], in_=sc_row,
                                func=ACTF.Exp, scale=1.0,
                                accum_out=rs2[:, 0:1])
                    else:
                        nc.scalar.activation(
                            out=expw[:, 512:1024], in_=sc_row,
                            func=ACTF.Exp, scale=1.0, accum_out=rs2[:, 1:2])

                if not own_tail:
                    return (e_ts, expw, rs2)

                # remaining tail: weight transposes c4-7, context tch4-7,
                # reduce, out
                wps_b = psum.tile([128, 4, 1], F32, tag="sc", bufs=1)
                for c in range(4):
                    nc.tensor.transpose(
                        wps_b[:, c, :],
                        expw[0:1, (c + 4) * 128:(c + 5) * 128],
                        one1,
                    )
                nc.vector.tensor_copy(out=wcol_pad[:, 4:8, 0:1], in_=wps_b)
                cp = epi["cp"]
                for half in range(2):
                    for tch in range(4, KC):
                        pos = 64 * (tch % 2) + 32 * half
                        nc.tensor.matmul(
                            cp[pos:pos + 32, :],
                            lhsT=wcol_pad[:, tch, :],
                            rhs=e_ts[tch][:, half * 512:(half + 1) * 512],
                            start=False,
                            stop=(tch >= KC - 2),
                            tile_position=(0, pos),
                        )
                epi_out(b, cp, (e_ts, expw, rs2))
                return None

            # ---------------- top-level software pipeline ----------------
            e_ts = emit_loads(0)
            eT = alloc_eT(0)
            for tt in range(TT):
                emit_transpose_tt(0, e_ts, eT, tt)

            prev_state = None
            for b in range(0, BL - 1):
                e_ts_n = emit_loads(b + 1)
                next_ctx = (e_ts_n, alloc_eT(b + 1))
                prev_ctx = (b - 1, prev_state) if prev_state is not None \
                    else None
                state = phase1(b, e_ts, eT, next_ctx, prev_ctx)
                prev_state = state
                e_ts, eT = next_ctx

            last_scs = None
            state = phase1(BL - 1, e_ts, eT, None, (BL - 2, prev_state),
                           final=True)
            # pipelined final tail: the second reduce rides the freed sc
            # bank so both reduces issue back-to-back, and each half's
            # weight transposes run between the two exps.
            e_ts_l, expw_l, rs2_l = state
            scr0 = psum.tile([1, 512], F32, tag="ctx", bufs=1)
            nc.tensor.matmul(scr0, lhsT=maskb_sb[:, 0:1], rhs=last_scs,
                             start=True, stop=True)
            scr1 = psum.tile([1, 512], F32, tag="sc", bufs=1)
            nc.tensor.matmul(scr1, lhsT=maskb_sb[:, 1:2], rhs=last_scs,
                             start=True, stop=True)
            nc.scalar.activation(out=expw_l[:, 0:512], in_=scr0,
                                 func=ACTF.Exp, scale=1.0,
                                 accum_out=rs2_l[:, 0:1])
            wps_a = psum.tile([128, 4, 1], F32, tag="ctx", bufs=1)
            for c in range(4):
                nc.tensor.transpose(wps_a[:, c, :],
                                    expw_l[0:1, c * 128:(c + 1) * 128], one1)
            nc.vector.tensor_copy(out=wcol_pad[:, 0:4, 0:1], in_=wps_a)
            nc.scalar.activation(out=expw_l[:, 512:1024], in_=scr1,
                                 func=ACTF.Exp, scale=1.0,
                                 accum_out=rs2_l[:, 1:2])
            wps_b = psum.tile([128, 4, 1], F32, tag="sc", bufs=1)
            for c in range(4):
                nc.tensor.transpose(
                    wps_b[:, c, :],
                    expw_l[0:1, (c + 4) * 128:(c + 5) * 128], one1)
            nc.vector.tensor_copy(out=wcol_pad[:, 4:8, 0:1], in_=wps_b)
            cp = epi_context(state)
            epi_out(BL - 1, cp, state)

    nc.compile()
    return nc


_NC_CACHE = None


def _get_nc():
    global _NC_CACHE
    if _NC_CACHE is None:
        _NC_CACHE = build_kernel()
    return _NC_CACHE


def make_in_maps(encoder_outputs, hidden_state, W1, b1, W2):
    enc = np.ascontiguousarray(np.asarray(encoder_outputs, np.float32))
    hid = np.ascontiguousarray(np.asarray(hidden_state, np.float32))
    W1 = np.asarray(W1, np.float32)
    b1 = np.asarray(b1, np.float32)
    W2 = np.asarray(W2, np.float32)

    bf16 = ml_dtypes.bfloat16
    f8 = ml_dtypes.float8_e4m3
    # cast to the HW e4m3 format, but ship the bytes under the e4m3fn
    # container dtype: the PJRT path rejects the IEEE f8E4M3 HLO type
    # while accepting f8E4M3FN, and bass's input check is fuzzy across
    # the two.
    w1a8 = np.ascontiguousarray(
        (W1[:H] * SW).reshape(KC, 128, H)).astype(f8).view(
            ml_dtypes.float8_e4m3fn)

    # per-j s2 scale: odd j's selu output is scaled by SW (DVE relu path)
    jscale = np.where(np.arange(JC) % 2 == 1, SW, 1.0).astype(np.float32)
    w2l = (W2[:, 0] * SELU_LAMBDA).reshape(JC, 128) / jscale[:, None]
    w2lp = np.zeros((128, JC, 32), bf16)
    w2lp[:, :, 0] = w2l.T.astype(bf16)

    # reduction masks: tg0/half0 partials live at partitions [0,32) and
    # [64,96) (real rows 0 and 64, zeros elsewhere), tg1/half1 at the
    # complement.
    m = np.zeros((128, 2), np.float32)
    m[0:32, 0] = 1.0
    m[64:96, 0] = 1.0
    m[32:64, 1] = 1.0
    m[96:128, 1] = 1.0

    # host-side hidden-state contribution: hb[b, :] = hid[b] @ W1[H:] + b1
    hb_all = hid[0] @ W1[H:] + b1                       # (B, H) f32
    ln_alpha = math.log(SELU_ALPHA)
    ln_sw = math.log(SW)

    in_maps = []
    for c in range(N_CORES):
        sl = slice(BL * c, BL * (c + 1))
        hb = hb_all[sl].reshape(BL, JC, 128).transpose(2, 1, 0)  # (128,JC,BL)
        # exp bias: hb + ln(alpha) (+ ln(SW) for odd j so e2 = SW*alpha*e^x)
        hbe = hb + ln_alpha + ln_sw * (np.arange(JC) % 2)[None, :, None]
        # relu bias: hb (ACT, even j) or SW*hb (DVE, odd j)
        hbr = hb * np.where(np.arange(JC) % 2 == 1, SW, 1.0)[None, :, None]
        in_maps.append({
            "enc": np.ascontiguousarray(enc[sl]).reshape(BL, TT, 128, H),
            "w1a8": w1a8,
            "w2lp": w2lp,
            "hbe": np.ascontiguousarray(hbe.astype(np.float32)),
            "hbr": np.ascontiguousarray(hbr.astype(np.float32)),
            "maskb": m.astype(bf16),
            "wcz": np.zeros((128, KC, 32), bf16),
        })
    return in_maps


def kernel(encoder_outputs, hidden_state, W1, b1, W2, b2):
    # b2 shifts every score equally; softmax is shift-invariant, so it is
    # deliberately unused.
    in_maps = make_in_maps(encoder_outputs, hidden_state, W1, b1, W2)
    nc = _get_nc()
    res = run_bass_kernel_spmd(nc, in_maps, core_ids=list(range(N_CORES)))
    out = np.empty((1, B, H), np.float32)
    for c in range(N_CORES):
        z = res.results[c]["zs"].sum(axis=1, keepdims=True)   # (BL, 1)
        p4 = res.results[c]["outp4"]                          # (BL, 4, 512)
        ctx = np.concatenate([p4[:, 0] + p4[:, 2],
                              p4[:, 1] + p4[:, 3]], axis=1)   # (BL, H)
        out[0, BL * c:BL * (c + 1)] = ctx / z
    return out
